# revision 14
# baseline (speedup 1.0000x reference)
"""Complex transformer block (LN->attn->LN->MLP, complex arithmetic) on 8 TRN2 cores.

Sharding: core c handles (batch b = c//2, sequence half = c%2). No collectives:
each core computes K/V over the full 1024-token sequence of its batch (the only
duplicated work) and queries/MLP over its own 512 tokens.

Layout: activations are feature-major [feature partition-blocks, tokens].
Complex tensors are realified as separate real/imag feature planes.

Attention path runs in fp8e4m3 with DoubleRow matmuls (2 K-planes per pass):
x, qkv/v/proj weights and the attention output are fp8; softmax scores/exp
stay bf16/f32. LayerNorm1 is folded into the qkv weights via per-token stat
rows (mu_r, mu_i, std appended to the contraction) with the rstd applied at
PSUM eviction - this keeps the LN off the critical path. The MLP runs in bf16
(fp8 there fails the error budget): LayerNorm2 is materialized once (xh2) and
gelu reads PSUM directly with a fused per-feature bias. Attention scores are
computed transposed ([t2, t1]) so softmax sums reduce via ones-matmuls, and V
is produced pre-transposed by swapping matmul operands. All weights are stored
host-side in the exact SBUF layout so every weight DMA is fully contiguous.
"""
import sys
sys.path.insert(0, "/opt/trn_rl_repo")

from contextlib import ExitStack

import ml_dtypes
import numpy as np

import concourse.bacc as bacc
import concourse.bass as bass
import concourse.mybir as mybir
import concourse.tile as tile
from concourse.bass_utils import run_bass_kernel_spmd

# Prefer the table set that covers the whole softmax chain (square+ln+exp)
# so the greedy act-table-load pass doesn't thrash sets on every block.
_orig_get_tables = bacc.get_activation_tables


def _reordered_tables(arch):
    t = _orig_get_tables(arch)
    keep = {"natural_log_exp_and_others", "gelu_and_others"}
    return {k: (v if k in keep else set()) for k, v in t.items()}


bacc.get_activation_tables = _reordered_tables

dt = mybir.dt
AF = mybir.ActivationFunctionType
ALU = mybir.AluOpType
DR = mybir.MatmulPerfMode.DoubleRow
BF16 = ml_dtypes.bfloat16
F8 = ml_dtypes.float8_e4m3

B, N, C, H, DH, HID = 4, 1024, 768, 12, 64, 3072
NCORES = 8
OWN = 512          # tokens per core
SCALE = DH ** -0.5
EPS = 1e-5


def round_fp32r(x):
    b = np.ascontiguousarray(x, dtype=np.float32).view(np.uint32)
    lsb = (b >> np.uint32(12)) & np.uint32(1)
    return ((b + np.uint32(0x7FF) + lsb) & np.uint32(0xFFFFF000)).view(np.float32)


# --------------------------------------------------------------------------
# device program
# --------------------------------------------------------------------------

def build_nc(debug=False):
    nc = bacc.Bacc(trn_type="TRN2", target_bir_lowering=False)
    f32 = dt.float32
    f32r = dt.float32r
    bf16 = dt.bfloat16
    f8 = dt.float8e4

    # ---- DRAM I/O ----
    x_r = nc.dram_tensor("x_r", [128, 12, N], f8, kind="ExternalInput")
    x_own = nc.dram_tensor("x_own", [128, 12, OWN], f32, kind="ExternalInput")
    w_qkv = nc.dram_tensor("w_qkv", [H, 128, 12, 384], f8, kind="ExternalInput")
    w_qkv_s = nc.dram_tensor("w_qkv_s", [H, 4, 384], f32r, kind="ExternalInput")
    w_v = nc.dram_tensor("w_v", [6, 128, 12, 256], f8, kind="ExternalInput")
    w_v_s = nc.dram_tensor("w_v_s", [4, 1536], f32r, kind="ExternalInput")
    w_proj = nc.dram_tensor("w_proj", [12, 128, 12, 128], f8, kind="ExternalInput")
    w_pb = nc.dram_tensor("w_pb", [128, 12], f32, kind="ExternalInput")
    w_fc1r = nc.dram_tensor("w_fc1r", [24, 128, 6, 128], bf16, kind="ExternalInput")
    w_fc1i = nc.dram_tensor("w_fc1i", [24, 128, 6, 128], bf16, kind="ExternalInput")
    w_fc1in = nc.dram_tensor("w_fc1in", [24, 128, 6, 128], bf16, kind="ExternalInput")
    w_fc1b = nc.dram_tensor("w_fc1b", [128, 24, 2], f32, kind="ExternalInput")
    w_fc2r = nc.dram_tensor("w_fc2r", [6, 128, 24, 128], bf16, kind="ExternalInput")
    w_fc2i = nc.dram_tensor("w_fc2i", [6, 128, 24, 128], bf16, kind="ExternalInput")
    w_fc2in = nc.dram_tensor("w_fc2in", [6, 128, 24, 128], bf16, kind="ExternalInput")
    w_fc2_s = nc.dram_tensor("w_fc2_s", [6, 4, 256], f32r, kind="ExternalInput")
    ones_col = nc.dram_tensor("ones_col", [128, 1], bf16, kind="ExternalInput")
    ones_ab8 = nc.dram_tensor("ones_ab8", [128, 4], f8, kind="ExternalInput")
    ones_s8 = nc.dram_tensor("ones_s8", [128, 1], f8, kind="ExternalInput")
    ones_ab = nc.dram_tensor("ones_ab", [128, 4], bf16, kind="ExternalInput")
    ones_s = nc.dram_tensor("ones_s", [128, 1], bf16, kind="ExternalInput")
    stat_one = nc.dram_tensor("stat_one", [4, OWN], f32r, kind="ExternalInput")
    ident8 = nc.dram_tensor("ident8", [8, 8], f32r, kind="ExternalInput")

    out_fm = nc.dram_tensor("out_fm", [12, 128, OWN], f32, kind="ExternalOutput")

    with tile.TileContext(nc) as tc, ExitStack() as top:
        consts = top.enter_context(tc.tile_pool(name="consts", bufs=1))
        t_ones_col = consts.tile([128, 1], bf16)
        t_ones_ab8 = consts.tile([128, 4], f8)
        t_ones_s8 = consts.tile([128, 1], f8)
        t_ones_ab = consts.tile([128, 4], bf16)
        t_ones_s = consts.tile([128, 1], bf16)
        t_stat_one = consts.tile([4, OWN], f32r)
        t_id8 = consts.tile([8, 8], f32r)
        t_pb = consts.tile([128, 12], f32)
        t_f1b = consts.tile([128, 24, 2], f32)
        t_eps = consts.tile([1, 1], f32)
        nc.sync.dma_start(t_ones_col[:], ones_col[:])
        nc.sync.dma_start(t_ones_ab8[:], ones_ab8[:])
        nc.sync.dma_start(t_ones_s8[:], ones_s8[:])
        nc.sync.dma_start(t_ones_ab[:], ones_ab[:])
        nc.sync.dma_start(t_ones_s[:], ones_s[:])
        nc.sync.dma_start(t_stat_one[:], stat_one[:])
        nc.sync.dma_start(t_id8[:], ident8[:])
        nc.sync.dma_start(t_pb[:], w_pb[:])
        nc.sync.dma_start(t_f1b[:], w_fc1b[:])
        nc.vector.memset(t_eps[:], EPS)

        poolR1 = top.enter_context(tc.tile_pool(name="poolR1", bufs=1))
        xr1 = poolR1.tile([128, 12, OWN], f32, name="xr1")

        with ExitStack() as es_x:
            poolX = es_x.enter_context(tc.tile_pool(name="poolX", bufs=1))
            x8 = poolX.tile([128, 12, N], f8, name="x8")
            pdram = es_x.enter_context(
                tc.tile_pool(name="pdram", bufs=1, space="DRAM"))
            rstd_dram = pdram.tile([1, N], f32, name="rstd_dram")
            stat1s = [poolX.tile([4, 512], f32r, name=f"stat1_{ch}")
                      for ch in range(2)]
            rstd_bc1s = [poolX.tile([128, 512], f32, name=f"rstd_bc1_{ch}")
                         for ch in range(2)]
            rstdT = poolX.tile([128, 8], f32, name="rstdT")
            for kb in range(12):
                nc.sync.dma_start(x8[:, kb, :], x_r[:, kb, :])

            # ---------------- phase A: LN1 stats over full sequence --------
            with ExitStack() as es_a:
                pa = es_a.enter_context(tc.tile_pool(name="pa_sb", bufs=3))
                pa_ps = es_a.enter_context(
                    tc.tile_pool(name="pa_ps", bufs=2, space="PSUM"))
                pa_sc = es_a.enter_context(tc.tile_pool(name="pa_sc", bufs=2))
                mu_pss = [pa_ps.tile([2, 512], f32, tag=f"mu{ch}",
                                     name=f"mu{ch}", bufs=1) for ch in range(2)]
                s_pss = [pa_ps.tile([1, 512], f32, tag=f"s{ch}",
                                    name=f"s{ch}", bufs=1) for ch in range(2)]
                for kb in range(12):
                    sq = pa.tile([128, N], f8, tag="sq", name=f"sq{kb}")
                    nc.scalar.activation(sq[:], x8[:, kb, :], AF.Square)
                    lhs = t_ones_ab8[:, 0:2] if kb < 6 else t_ones_ab8[:, 2:4]
                    for ch in range(2):
                        sl = slice(ch * 512, ch * 512 + 512)
                        nc.tensor.matmul(mu_pss[ch][:], lhs, x8[:, kb, sl],
                                         start=(kb == 0), stop=(kb == 11))
                        nc.tensor.matmul(s_pss[ch][:], t_ones_s8[:], sq[:, sl],
                                         start=(kb == 0), stop=(kb == 11))
                for ch in range(2):
                    sl = slice(ch * 512, ch * 512 + 512)
                    mu_ps = mu_pss[ch]
                    s_ps = s_pss[ch]
                    # var = S - mu_r^2 - mu_i^2 ; std = exp(.5 ln(var+eps))
                    mu_sb = pa_sc.tile([2, 512], f32, tag="musb", name=f"musb{ch}")
                    mu_fl = pa_sc.tile([1, 2, 512], f32, tag="mufl", name=f"mufl{ch}")
                    var = pa_sc.tile([1, 512], f32, tag="var", name=f"var{ch}")
                    lnv = pa_sc.tile([1, 512], f32, tag="lnv", name=f"lnv{ch}")
                    s_c = pa_sc.tile([1, 512], f32, tag="sc_", name=f"sc_{ch}")
                    nc.vector.tensor_scalar(mu_sb[:], mu_ps[:], 1.0 / C, None,
                                            op0=ALU.mult)
                    nc.vector.tensor_scalar(s_c[:], s_ps[:], 1.0 / C, None,
                                            op0=ALU.mult)
                    nc.sync.dma_start(mu_fl[:, 0, :], mu_sb[0:1, :])
                    nc.sync.dma_start(mu_fl[:, 1, :], mu_sb[1:2, :])
                    sq_mu = pa_sc.tile([1, 2, 512], f32, tag="sqmu", name=f"sqmu{ch}")
                    nc.vector.tensor_tensor(sq_mu[:], mu_fl[:], mu_fl[:],
                                            op=ALU.mult)
                    nc.vector.tensor_tensor(var[:], s_c[:], sq_mu[:, 0, :],
                                            op=ALU.subtract)
                    nc.vector.tensor_tensor(var[:], var[:], sq_mu[:, 1, :],
                                            op=ALU.subtract)
                    nc.scalar.activation(lnv[:], var[:], AF.Ln, bias=t_eps[:])
                    # stats rows: 0=mu_r 1=mu_i 2=std
                    nc.vector.tensor_copy(stat1s[ch][0:2, :], mu_sb[:])
                    std_row = pa_sc.tile([1, 512], f32r, tag="stdr", name=f"stdr{ch}")
                    nc.scalar.activation(std_row[:], lnv[:], AF.Exp, scale=0.5)
                    nc.sync.dma_start(stat1s[ch][2:3, :], std_row[:])
                    rstd_row = pa_sc.tile([1, 512], f32r, tag="rst", name=f"rst{ch}")
                    nc.scalar.activation(rstd_row[:], lnv[:], AF.Exp, scale=-0.5)
                    nc.sync.dma_start(rstd_dram[:, sl], rstd_row[:].bitcast(f32))
                    nc.gpsimd.partition_broadcast(
                        rstd_bc1s[ch][:], rstd_row[:].bitcast(f32))
                # rstd transposed: rstdT[p, t2b] = rstd[t2b*128 + p]
                rstd8 = pa_sc.tile([8, 128], f32, tag="r8", name="rstd8")
                nc.sync.dma_start(
                    rstd8[:], rstd_dram[:].rearrange("o (a b) -> (o a) b", a=8))
                rstdT_ps = pa_ps.tile([128, 8], f32, tag="rtps", name="rtps")
                nc.tensor.transpose(rstdT_ps[:], rstd8[:], t_id8[:].bitcast(f32))
                nc.vector.tensor_copy(rstdT[:], rstdT_ps[:])

            # ---------------- phase BC: qkv + attention per head ----------
            es_attn = ExitStack()
            attnp = es_attn.enter_context(tc.tile_pool(name="attnp", bufs=1))
            attn = attnp.tile([128, 12, OWN], f8, name="attn")
            es_b = ExitStack()
            pq = es_b.enter_context(tc.tile_pool(name="pq", bufs=1))
            pk = es_b.enter_context(tc.tile_pool(name="pk", bufs=1))
            pvt = es_b.enter_context(tc.tile_pool(name="pvt", bufs=2))
            pwv = es_b.enter_context(tc.tile_pool(name="pwv", bufs=1))
            pwq = es_b.enter_context(tc.tile_pool(name="pwq", bufs=2))
            pet = es_b.enter_context(tc.tile_pool(name="pet", bufs=5))
            psc = es_b.enter_context(tc.tile_pool(name="psc", bufs=4))
            prd = es_b.enter_context(tc.tile_pool(name="prd", bufs=2))
            ps_rot = es_b.enter_context(
                tc.tile_pool(name="ps_rot", bufs=2, space="PSUM"))
            ps_sc = es_b.enter_context(
                tc.tile_pool(name="ps_sc", bufs=2, space="PSUM"))
            ps_acc = es_b.enter_context(
                tc.tile_pool(name="ps_acc", bufs=2, space="PSUM"))
            pdram_rd = es_b.enter_context(
                tc.tile_pool(name="pdram_rd", bufs=2, space="DRAM"))
            vt_pair = None
            et_fifo = []
            acc_ps = {}
            LAG = 6

            def emit_avden(ent):
                h2, t2b2, et2, vt2 = ent
                slot2 = h2 % 2
                if t2b2 == 0:
                    acc_ps[h2] = (
                        ps_acc.tile([128, OWN], f32, tag="av", name=f"av{h2}",
                                    bufs=1),
                        ps_acc.tile([1, OWN], f32, tag="den", name=f"den{h2}",
                                    bufs=1),
                    )
                av2, den2 = acc_ps[h2]
                nc.tensor.matmul(den2[:], t_ones_col[:], et2,
                                 start=(t2b2 == 0), stop=(t2b2 == 7))
                dsl2 = slice(slot2 * 128, slot2 * 128 + 128)
                nc.tensor.matmul(av2[:], vt2[:, t2b2, dsl2], et2,
                                 start=(t2b2 == 0), stop=(t2b2 == 7))
                if t2b2 == 7:
                    den_sb = prd.tile([1, OWN], f32, tag="den_sb",
                                      name=f"dsb{h2}", bufs=1)
                    nc.vector.tensor_copy(den_sb[:], den2[:])
                    den_dram = pdram_rd.tile([1, OWN], f32, tag="dend",
                                             name=f"dend{h2}")
                    nc.sync.dma_start(den_dram[:], den_sb[:])
                    den_sp = prd.tile([128, 4], f32, tag="den_sp",
                                      name=f"dsp{h2}", bufs=1)
                    nc.sync.dma_start(
                        den_sp[:],
                        den_dram[:].rearrange("o (a b) -> (o a) b", a=128))
                    rd_sp = prd.tile([128, 4], f32, tag="rd_sp",
                                     name=f"rsp{h2}", bufs=1)
                    nc.vector.reciprocal(rd_sp[:], den_sp[:])
                    rd_dram = pdram_rd.tile([1, OWN], f32, tag="rdd",
                                            name=f"rdd{h2}")
                    nc.sync.dma_start(
                        rd_dram[:].rearrange("o (a b) -> (o a) b", a=128),
                        rd_sp[:])
                    rd_bc = prd.tile([128, OWN], f32, tag="rd_bc",
                                     name=f"rdbc{h2}", bufs=1)
                    rd_bcast_ap = bass.AP(tensor=rd_dram.tensor,
                                          offset=rd_dram[:].offset,
                                          ap=[[0, 128]] + rd_dram[:].ap[1:])
                    nc.sync.dma_start(rd_bc[:], rd_bcast_ap)
                    nc.vector.tensor_tensor(attn[:, h2, :], av2[:], rd_bc[:],
                                            op=ALU.mult)
                    del acc_ps[h2]

            for h in range(H):
                pair, slot = divmod(h, 2)
                # qkv for head h: q1=[q_r;-q_i], q3=[q_i;q_r], k=[k_r;k_i]
                q_t = pq.tile([128, 2, OWN], bf16, tag="q", name=f"q{h}")
                k_t = pk.tile([128, N], bf16, tag="k", name=f"k{h}")
                wqkv_t = pwq.tile([128, 12, 384], f8, tag="wqkv",
                                  name=f"wqkv{h}")
                wqs_t = pwq.tile([4, 384], f32r, tag="wqs", name=f"wqs{h}")
                nc.sync.dma_start(wqkv_t[:], w_qkv[h])
                nc.sync.dma_start(wqs_t[:], w_qkv_s[h])
                q1_ps = ps_rot.tile([128, OWN], f32, tag="rot", name=f"q1ps{h}")
                q3_ps = ps_rot.tile([128, OWN], f32, tag="rot", name=f"q3ps{h}")
                for p in range(6):
                    kp = slice(2 * p, 2 * p + 2)
                    st = (p == 0)
                    nc.tensor.matmul(q1_ps[:], wqkv_t[:, kp, 0:128],
                                     x8[:, kp, 0:OWN], start=st, stop=False,
                                     perf_mode=DR)
                    nc.tensor.matmul(q3_ps[:], wqkv_t[:, kp, 128:256],
                                     x8[:, kp, 0:OWN], start=st, stop=False,
                                     perf_mode=DR)
                nc.tensor.matmul(q1_ps[:], wqs_t[:, 0:128], stat1s[0][:],
                                 start=False, stop=True)
                nc.tensor.matmul(q3_ps[:], wqs_t[:, 128:256], stat1s[0][:],
                                 start=False, stop=True)
                nc.vector.tensor_tensor(q_t[:, 0, :], q1_ps[:],
                                        rstd_bc1s[0][:], op=ALU.mult)
                nc.vector.tensor_tensor(q_t[:, 1, :], q3_ps[:],
                                        rstd_bc1s[0][:], op=ALU.mult)
                k0_ps = ps_rot.tile([128, 512], f32, tag="rot", name=f"k0ps{h}")
                k1_ps = ps_rot.tile([128, 512], f32, tag="rot", name=f"k1ps{h}")
                for p in range(6):
                    kp = slice(2 * p, 2 * p + 2)
                    st = (p == 0)
                    nc.tensor.matmul(k0_ps[:], wqkv_t[:, kp, 256:384],
                                     x8[:, kp, 0:512], start=st, stop=False,
                                     perf_mode=DR)
                    nc.tensor.matmul(k1_ps[:], wqkv_t[:, kp, 256:384],
                                     x8[:, kp, 512:N], start=st, stop=False,
                                     perf_mode=DR)
                nc.tensor.matmul(k0_ps[:], wqs_t[:, 256:384], stat1s[0][:],
                                 start=False, stop=True)
                nc.tensor.matmul(k1_ps[:], wqs_t[:, 256:384], stat1s[1][:],
                                 start=False, stop=True)
                nc.vector.tensor_tensor(k_t[:, 0:512], k0_ps[:],
                                        rstd_bc1s[0][:], op=ALU.mult)
                nc.vector.tensor_tensor(k_t[:, 512:N], k1_ps[:],
                                        rstd_bc1s[1][:], op=ALU.mult)
                if slot == 0:
                    # V^T for this head pair: [t2, d] via swapped operands
                    wv_t = pwv.tile([128, 12, 256], f8, tag="wv",
                                    name=f"wv{pair}")
                    wv_s = pwv.tile([4, 256], f32r, tag="wvs",
                                    name=f"wvs{pair}")
                    csl = slice(pair * 256, pair * 256 + 256)
                    nc.sync.dma_start(wv_t[:], w_v[pair])
                    nc.sync.dma_start(wv_s[:], w_v_s[:, csl])
                    vt_pair = pvt.tile([128, 8, 256], bf16, tag="vt",
                                       name=f"vt{pair}")
                    for t2b in range(8):
                        t2s = slice(t2b * 128, t2b * 128 + 128)
                        vt_ps = ps_rot.tile([128, 256], f32, tag="rot",
                                            name=f"vtps{pair}_{t2b}")
                        for p in range(6):
                            kp = slice(2 * p, 2 * p + 2)
                            nc.tensor.matmul(vt_ps[:], x8[:, kp, t2s],
                                             wv_t[:, kp, :],
                                             start=(p == 0), stop=False,
                                             perf_mode=DR)
                        st1 = stat1s[t2b // 4]
                        t2l = slice((t2b % 4) * 128, (t2b % 4) * 128 + 128)
                        nc.tensor.matmul(vt_ps[:], st1[:, t2l], wv_s[:],
                                         start=False, stop=True)
                        nc.vector.tensor_scalar(
                            vt_pair[:, t2b, :], vt_ps[:],
                            rstdT[:, t2b:t2b + 1], None, op0=ALU.mult)
                # scores + exp chain, batched over block pairs;
                # den/av matmuls lag by LAG sub-blocks
                for t2p in range(4):
                    t2s0 = slice(t2p * 256, t2p * 256 + 128)
                    t2s1 = slice(t2p * 256 + 128, t2p * 256 + 256)
                    sr_pair = ps_sc.tile([128, 2, OWN], f32, tag="scp",
                                         name=f"srp{h}_{t2p}")
                    si_pair = ps_sc.tile([128, 2, OWN], f32, tag="scp",
                                         name=f"sip{h}_{t2p}")
                    nc.tensor.matmul(sr_pair[:, 0, :], k_t[:, t2s0],
                                     q_t[:, 0, :], start=True, stop=True)
                    nc.tensor.matmul(si_pair[:, 0, :], k_t[:, t2s0],
                                     q_t[:, 1, :], start=True, stop=True)
                    nc.tensor.matmul(sr_pair[:, 1, :], k_t[:, t2s1],
                                     q_t[:, 0, :], start=True, stop=True)
                    nc.tensor.matmul(si_pair[:, 1, :], k_t[:, t2s1],
                                     q_t[:, 1, :], start=True, stop=True)
                    sqr = psc.tile([128, 2, OWN], f32, tag="sqr",
                                   name=f"sqr{h}_{t2p}")
                    sqi = psc.tile([128, 2, OWN], f32, tag="sqi",
                                   name=f"sqi{h}_{t2p}")
                    nc.scalar.activation(sqr[:], sr_pair[:], AF.Square)
                    nc.scalar.activation(sqi[:], si_pair[:], AF.Square)
                    # in-place chain on sqr: m2 -> ln -> 0.5ln -> mag -> exp
                    nc.gpsimd.tensor_tensor(sqr[:], sqr[:], sqi[:],
                                            op=ALU.add)
                    nc.scalar.activation(sqr[:], sqr[:], AF.Ln)
                    nc.scalar.activation(sqr[:], sqr[:], AF.Exp, scale=0.5)
                    et = pet.tile([128, 2, OWN], bf16, tag="et",
                                  name=f"et{h}_{t2p}")
                    nc.scalar.activation(et[:], sqr[:], AF.Exp)
                    for sub in range(2):
                        et_fifo.append((h, t2p * 2 + sub, et[:, sub, :],
                                        vt_pair))
                        while len(et_fifo) > LAG:
                            emit_avden(et_fifo.pop(0))
            for ent in et_fifo:
                emit_avden(ent)
            et_fifo.clear()
            es_b.close()

            # ------------- phase D: proj + residual --------------------
            nc.sync.dma_start(xr1[:], x_own[:])
            for opb in range(12):
                nc.vector.tensor_scalar(xr1[:, opb, :], xr1[:, opb, :],
                                        t_pb[:, opb:opb + 1], None,
                                        op0=ALU.add)
            r1r = poolR1.tile([128, 12, OWN], bf16, name="r1r")
            with ExitStack() as es_d:
                pwp = es_d.enter_context(tc.tile_pool(name="pwp", bufs=3))
                ps_d = es_d.enter_context(
                    tc.tile_pool(name="ps_d", bufs=4, space="PSUM"))
                for opb in range(12):
                    wp_t = pwp.tile([128, 12, 128], f8, tag="wp",
                                    name=f"wp{opb}")
                    nc.sync.dma_start(wp_t[:], w_proj[opb])
                    pr_ps = ps_d.tile([128, OWN], f32, tag="pr",
                                      name=f"prps{opb}")
                    for p in range(6):
                        kp = slice(2 * p, 2 * p + 2)
                        nc.tensor.matmul(pr_ps[:], wp_t[:, kp, :],
                                         attn[:, kp, :],
                                         start=(p == 0), stop=(p == 5),
                                         perf_mode=DR)
                    nc.vector.tensor_tensor(xr1[:, opb, :], pr_ps[:],
                                            xr1[:, opb, :], op=ALU.add)
                    nc.vector.tensor_copy(r1r[:, opb, :], xr1[:, opb, :])
            es_attn.close()

        # ---------------- phase E: LN2 stats + normalized r1 --------------
        poolE = top.enter_context(tc.tile_pool(name="poolE", bufs=1))
        xh2 = poolE.tile([128, 12, OWN], bf16, name="xh2")
        with ExitStack() as es_e:
            pe = es_e.enter_context(tc.tile_pool(name="pe_sb", bufs=1))
            pdram2 = es_e.enter_context(
                tc.tile_pool(name="pdram2", bufs=1, space="DRAM"))
            pe_ps = es_e.enter_context(
                tc.tile_pool(name="pe_ps", bufs=2, space="PSUM"))
            sq2s = []
            for kb in range(12):
                sq2 = pe.tile([128, OWN], bf16, tag="sq2", name=f"sq2_{kb}", bufs=12)
                nc.scalar.activation(sq2[:], r1r[:, kb, :], AF.Square)
                sq2s.append(sq2)
            mu2_ps = pe_ps.tile([2, OWN], f32, tag="mu2", name="mu2")
            s2_ps = pe_ps.tile([1, OWN], f32, tag="s2", name="s2")
            for kb in range(12):
                lhs = t_ones_ab[:, 0:2] if kb < 6 else t_ones_ab[:, 2:4]
                nc.tensor.matmul(mu2_ps[:], lhs, r1r[:, kb, :],
                                 start=(kb == 0), stop=(kb == 11))
                nc.tensor.matmul(s2_ps[:], t_ones_s[:], sq2s[kb][:],
                                 start=(kb == 0), stop=(kb == 11))
            mu2_sb = pe.tile([2, OWN], f32, tag="emusb", name="emusb")
            mu2_fl = pe.tile([1, 2, OWN], f32, tag="emufl", name="emufl")
            var = pe.tile([1, OWN], f32, tag="evar", name="evar")
            lnv = pe.tile([1, OWN], f32, tag="elnv", name="elnv")
            s2_c = pe.tile([1, OWN], f32, tag="es2c", name="es2c")
            nc.vector.tensor_scalar(mu2_sb[:], mu2_ps[:], 1.0 / C, None,
                                    op0=ALU.mult)
            nc.vector.tensor_scalar(s2_c[:], s2_ps[:], 1.0 / C, None,
                                    op0=ALU.mult)
            nc.sync.dma_start(mu2_fl[:, 0, :], mu2_sb[0:1, :])
            nc.sync.dma_start(mu2_fl[:, 1, :], mu2_sb[1:2, :])
            sq_mu2 = pe.tile([1, 2, OWN], f32, tag="esqmu", name="esqmu")
            nc.vector.tensor_tensor(sq_mu2[:], mu2_fl[:], mu2_fl[:], op=ALU.mult)
            nc.vector.tensor_tensor(var[:], s2_c[:], sq_mu2[:, 0, :],
                                    op=ALU.subtract)
            nc.vector.tensor_tensor(var[:], var[:], sq_mu2[:, 1, :],
                                    op=ALU.subtract)
            nc.scalar.activation(lnv[:], var[:], AF.Ln, bias=t_eps[:])
            rstd2_row = pe.tile([1, OWN], f32, tag="ers", name="ers")
            nc.scalar.activation(rstd2_row[:], lnv[:], AF.Exp, scale=-0.5)
            mu2r_bc = pe.tile([128, OWN], f32, tag="m2rbc", name="m2rbc")
            mu2i_bc = pe.tile([128, OWN], f32, tag="m2ibc", name="m2ibc")
            rstd2_bc = pe.tile([128, OWN], f32, tag="r2bc", name="r2bc")
            nc.gpsimd.partition_broadcast(mu2r_bc[:], mu2_fl[:, 0, :])
            nc.gpsimd.partition_broadcast(mu2i_bc[:], mu2_fl[:, 1, :])
            nc.gpsimd.partition_broadcast(rstd2_bc[:], rstd2_row[:])
            for kb in range(12):
                mbc = mu2r_bc if kb < 6 else mu2i_bc
                nc.vector.tensor_tensor(xh2[:, kb, :], r1r[:, kb, :],
                                        mbc[:], op=ALU.subtract)
                nc.vector.tensor_tensor(xh2[:, kb, :], xh2[:, kb, :],
                                        rstd2_bc[:], op=ALU.mult)

        # ---------------- phase F: MLP, single 512-token pass -------------
        with ExitStack() as es_f:
            ph = es_f.enter_context(tc.tile_pool(name="ph", bufs=1))
            pw1 = es_f.enter_context(tc.tile_pool(name="pw1", bufs=6))
            pw2 = es_f.enter_context(tc.tile_pool(name="pw2", bufs=4))
            pout = es_f.enter_context(tc.tile_pool(name="pout", bufs=2))
            ps_f = es_f.enter_context(
                tc.tile_pool(name="ps_f", bufs=4, space="PSUM"))
            h_t = ph.tile([128, 48, OWN], bf16, name="h_t")
            for Cb in range(24):
                w1r_t = pw1.tile([128, 6, 128], bf16, tag="w1r",
                                 name=f"w1r{Cb}")
                w1i_t = pw1.tile([128, 6, 128], bf16, tag="w1i",
                                 name=f"w1i{Cb}")
                w1in_t = pw1.tile([128, 6, 128], bf16, tag="w1in",
                                  name=f"w1in{Cb}")
                nc.sync.dma_start(w1r_t[:], w_fc1r[Cb])
                nc.sync.dma_start(w1i_t[:], w_fc1i[Cb])
                nc.sync.dma_start(w1in_t[:], w_fc1in[Cb])
                hr_ps = ps_f.tile([128, OWN], f32, tag="fps",
                                  name=f"hrps{Cb}")
                hi_ps = ps_f.tile([128, OWN], f32, tag="fps",
                                  name=f"hips{Cb}")
                for kb in range(6):
                    st = (kb == 0)
                    nc.tensor.matmul(hr_ps[:], w1r_t[:, kb, :],
                                     xh2[:, kb, :], start=st, stop=False)
                    nc.tensor.matmul(hi_ps[:], w1i_t[:, kb, :],
                                     xh2[:, kb, :], start=st, stop=False)
                for kb in range(6):
                    lst = (kb == 5)
                    nc.tensor.matmul(hr_ps[:], w1in_t[:, kb, :],
                                     xh2[:, 6 + kb, :], start=False,
                                     stop=lst)
                    nc.tensor.matmul(hi_ps[:], w1r_t[:, kb, :],
                                     xh2[:, 6 + kb, :], start=False,
                                     stop=lst)
                nc.scalar.activation(h_t[:, Cb, :], hr_ps[:], AF.Gelu,
                                     bias=t_f1b[:, Cb, 0:1])
                nc.scalar.activation(h_t[:, 24 + Cb, :], hi_ps[:], AF.Gelu,
                                     bias=t_f1b[:, Cb, 1:2])
            for j in range(6):
                w2r_t = pw2.tile([128, 24, 128], bf16, tag="w2r",
                                 name=f"w2r{j}")
                w2i_t = pw2.tile([128, 24, 128], bf16, tag="w2i",
                                 name=f"w2i{j}")
                w2in_t = pw2.tile([128, 24, 128], bf16, tag="w2in",
                                  name=f"w2in{j}")
                w2s_t = pw2.tile([4, 256], f32r, tag="w2s",
                                 name=f"w2s{j}")
                nc.sync.dma_start(w2r_t[:], w_fc2r[j])
                nc.sync.dma_start(w2i_t[:], w_fc2i[j])
                nc.sync.dma_start(w2in_t[:], w_fc2in[j])
                nc.sync.dma_start(w2s_t[:], w_fc2_s[j])
                or_ps = ps_f.tile([128, OWN], f32, tag="fps",
                                  name=f"orps{j}")
                oi_ps = ps_f.tile([128, OWN], f32, tag="fps",
                                  name=f"oips{j}")
                for kb in range(24):
                    st = (kb == 0)
                    nc.tensor.matmul(or_ps[:], w2r_t[:, kb, :], h_t[:, kb, :],
                                     start=st, stop=False)
                    nc.tensor.matmul(oi_ps[:], w2i_t[:, kb, :], h_t[:, kb, :],
                                     start=st, stop=False)
                for kb in range(24):
                    nc.tensor.matmul(or_ps[:], w2in_t[:, kb, :],
                                     h_t[:, 24 + kb, :],
                                     start=False, stop=False)
                    nc.tensor.matmul(oi_ps[:], w2r_t[:, kb, :],
                                     h_t[:, 24 + kb, :],
                                     start=False, stop=False)
                nc.tensor.matmul(or_ps[:], w2s_t[:, 0:128],
                                 t_stat_one[:], start=False, stop=True)
                nc.tensor.matmul(oi_ps[:], w2s_t[:, 128:256],
                                 t_stat_one[:], start=False, stop=True)
                o_r = pout.tile([128, OWN], f32, tag="o", name=f"or{j}")
                o_i = pout.tile([128, OWN], f32, tag="o", name=f"oi{j}")
                nc.vector.tensor_tensor(o_r[:], or_ps[:], xr1[:, j, :],
                                        op=ALU.add)
                nc.vector.tensor_tensor(o_i[:], oi_ps[:], xr1[:, 6 + j, :],
                                        op=ALU.add)
                nc.sync.dma_start(out_fm[j], o_r[:])
                nc.sync.dma_start(out_fm[6 + j], o_i[:])
    nc.compile()
    return nc


# --------------------------------------------------------------------------
# host side
# --------------------------------------------------------------------------

def _cx(a):
    return a[..., 0].astype(np.float64) + 1j * a[..., 1].astype(np.float64)


def _kcols(Wp, wsum, wb, plane, scale=1.0):
    """K-profile [1539, m] for output features with complex weight rows Wp
    [m, 768], LN fold sums wsum [m], bias-column wb [m]. K rows: xr(768),
    xi(768), mu_r, mu_i, std."""
    m = Wp.shape[0]
    out = np.zeros((1539, m), np.float64)
    if plane == "r":
        out[0:768] = Wp.real.T
        out[768:1536] = -Wp.imag.T
        out[1536] = -wsum.real
        out[1537] = wsum.imag
        out[1538] = wb.real
    else:
        out[0:768] = Wp.imag.T
        out[768:1536] = Wp.real.T
        out[1536] = -wsum.imag
        out[1537] = -wsum.real
        out[1538] = wb.imag
    return out * scale


def _bf(a):
    return np.ascontiguousarray(a).astype(BF16)


def _f8(a):
    return np.ascontiguousarray(a).astype(F8)


def _pmajor(a):
    """[kb, 128, n] -> [128, kb, n] partition-major contiguous."""
    return np.ascontiguousarray(np.transpose(a, (1, 0, 2)))


def _prep_weights(inputs):
    n1 = _cx(inputs["n1_w"]); b1 = _cx(inputs["n1_b"])
    n2 = _cx(inputs["n2_w"]); b2 = _cx(inputs["n2_b"])
    Wqkv = _cx(inputs["qkv_w"])          # [2304, 768]
    Wp = _cx(inputs["proj_w"])           # [768, 768]
    bp = _cx(inputs["proj_b"])           # [768]
    W1 = _cx(inputs["fc1_w"])            # [3072, 768]
    bf1 = _cx(inputs["fc1_b"])           # [3072]
    W2 = _cx(inputs["fc2_w"])            # [768, 3072]
    bf2 = _cx(inputs["fc2_b"])           # [768]

    d = {}
    # ---- qkv (LN1-folded) ----
    Wq, Wk, Wv = Wqkv[0:768], Wqkv[768:1536], Wqkv[1536:2304]

    def fold1(W):
        Wf = W * n1[None, :]
        return Wf, Wf.sum(1), W @ b1

    w_qkv = np.zeros((H, 128, 12, 384), F8)
    w_qkv_s = np.zeros((H, 4, 384), np.float32)
    for h in range(H):
        rows = slice(h * DH, (h + 1) * DH)
        Qf, Qs, Qb = fold1(Wq[rows])
        Kf, Ks, Kb_ = fold1(Wk[rows])
        q1 = np.hstack([_kcols(Qf, Qs, Qb, "r", SCALE),
                        _kcols(Qf, Qs, Qb, "i", -SCALE)])
        q3 = np.hstack([_kcols(Qf, Qs, Qb, "i", SCALE),
                        _kcols(Qf, Qs, Qb, "r", SCALE)])
        kk = np.hstack([_kcols(Kf, Ks, Kb_, "r"), _kcols(Kf, Ks, Kb_, "i")])
        blk = np.hstack([q1, q3, kk]).astype(np.float32)       # [1539, 384]
        w_qkv[h] = _f8(_pmajor(blk[0:1536].reshape(12, 128, 384)))
        w_qkv_s[h, 0:3] = blk[1536:1539]
    d["w_qkv"] = w_qkv
    d["w_qkv_s"] = round_fp32r(w_qkv_s)

    # ---- v (LN1-folded), rhs layout; cols: pair*256+slot*128+plane*64+dh
    wv_full = np.zeros((1539, 1536), np.float64)
    for h in range(H):
        rows = slice(h * DH, (h + 1) * DH)
        Vf, Vs, Vb = fold1(Wv[rows])
        base = h * 128
        wv_full[:, base:base + 64] = _kcols(Vf, Vs, Vb, "r")
        wv_full[:, base + 64:base + 128] = _kcols(Vf, Vs, Vb, "i")
    w_v = np.zeros((6, 128, 12, 256), F8)
    for pair in range(6):
        csl = slice(pair * 256, pair * 256 + 256)
        w_v[pair] = _f8(_pmajor(wv_full[0:1536, csl].reshape(12, 128, 256)))
    d["w_v"] = w_v
    wvs = np.zeros((4, 1536), np.float32)
    wvs[0:3] = wv_full[1536:1539]
    d["w_v_s"] = round_fp32r(wvs)

    # ---- proj; K rows = attn features: per head [a_r(64); a_i(64)] ----
    w_proj = np.zeros((12, 128, 12, 128), F8)
    w_pb = np.zeros((128, 12), np.float32)
    for opb in range(12):
        plane = "r" if opb < 6 else "i"
        orow = slice((opb % 6) * 128, (opb % 6) * 128 + 128)
        Wpo = Wp[orow]                               # [128, 768] complex
        prof = np.zeros((1536, 128), np.float64)
        for hh in range(H):
            cols = slice(hh * DH, (hh + 1) * DH)
            if plane == "r":
                prof[hh * 128:hh * 128 + 64] = Wpo.real[:, cols].T
                prof[hh * 128 + 64:hh * 128 + 128] = -Wpo.imag[:, cols].T
            else:
                prof[hh * 128:hh * 128 + 64] = Wpo.imag[:, cols].T
                prof[hh * 128 + 64:hh * 128 + 128] = Wpo.real[:, cols].T
        w_proj[opb] = _f8(_pmajor(prof.reshape(12, 128, 128)))
        w_pb[:, opb] = (bp.real if plane == "r" else bp.imag)[orow]
    d["w_proj"] = w_proj
    d["w_pb"] = w_pb

    # ---- fc1 (gain-folded; bias separate; LN2 applied via xh2) ----
    W1f = W1 * n2[None, :]
    W1b = W1 @ b2 + bf1
    w_fc1r = np.zeros((24, 128, 6, 128), BF16)
    w_fc1i = np.zeros((24, 128, 6, 128), BF16)
    w_fc1in = np.zeros((24, 128, 6, 128), BF16)
    w_fc1b = np.zeros((128, 24, 2), np.float32)
    for Cb in range(24):
        orow = slice(Cb * 128, (Cb + 1) * 128)
        tr = np.zeros((6, 128, 128), np.float64)
        ti = np.zeros((6, 128, 128), np.float64)
        for kb in range(6):
            icol = slice(kb * 128, (kb + 1) * 128)
            tr[kb] = W1f.real[orow, icol].T
            ti[kb] = W1f.imag[orow, icol].T
        w_fc1r[Cb] = _bf(_pmajor(tr))
        w_fc1i[Cb] = _bf(_pmajor(ti))
        w_fc1in[Cb] = _bf(_pmajor(-ti))
        w_fc1b[:, Cb, 0] = W1b.real[orow]
        w_fc1b[:, Cb, 1] = W1b.imag[orow]
    d["w_fc1r"] = w_fc1r
    d["w_fc1i"] = w_fc1i
    d["w_fc1in"] = w_fc1in
    d["w_fc1b"] = w_fc1b

    # ---- fc2 (plain + bias) ----
    w_fc2r = np.zeros((6, 128, 24, 128), BF16)
    w_fc2i = np.zeros((6, 128, 24, 128), BF16)
    w_fc2in = np.zeros((6, 128, 24, 128), BF16)
    w_fc2_s = np.zeros((6, 4, 256), np.float32)
    for j in range(6):
        orow = slice(j * 128, (j + 1) * 128)
        tr = np.zeros((24, 128, 128), np.float64)
        ti = np.zeros((24, 128, 128), np.float64)
        for kb in range(24):
            icol = slice(kb * 128, (kb + 1) * 128)
            tr[kb] = W2.real[orow, icol].T
            ti[kb] = W2.imag[orow, icol].T
        w_fc2r[j] = _bf(_pmajor(tr))
        w_fc2i[j] = _bf(_pmajor(ti))
        w_fc2in[j] = _bf(_pmajor(-ti))
        w_fc2_s[j, 0, 0:128] = bf2.real[orow]
        w_fc2_s[j, 0, 128:256] = bf2.imag[orow]
    d["w_fc2r"] = w_fc2r
    d["w_fc2i"] = w_fc2i
    d["w_fc2in"] = w_fc2in
    d["w_fc2_s"] = round_fp32r(w_fc2_s)

    # ---- consts ----
    d["ones_col"] = np.ones((128, 1), BF16)
    oab = np.zeros((128, 4), np.float32)
    oab[:, 0] = 1.0
    oab[:, 3] = 1.0
    d["ones_ab8"] = oab.astype(F8)
    d["ones_s8"] = np.ones((128, 1), F8)
    d["ones_ab"] = oab.astype(BF16)
    d["ones_s"] = np.ones((128, 1), BF16)
    so = np.zeros((4, OWN), np.float32)
    so[0] = 1.0
    d["stat_one"] = so
    d["ident8"] = np.eye(8, dtype=np.float32)
    return d


_NC_CACHE = {}


def kernel(**inputs):
    debug = bool(inputs.pop("_debug", False))
    if debug not in _NC_CACHE:
        _NC_CACHE[debug] = build_nc(debug=debug)
    nc = _NC_CACHE[debug]

    shared = _prep_weights(inputs)
    x = np.asarray(inputs["x"], np.float32)          # [B, N, C, 2]

    in_maps = []
    for c in range(NCORES):
        b, half = divmod(c, 2)
        xr_ = x[b, :, :, 0].T                        # [768, 1024]
        xi_ = x[b, :, :, 1].T
        stack = np.concatenate([xr_, xi_], 0)        # [1536, 1024]
        if half == 1:
            stack = np.concatenate([stack[:, OWN:], stack[:, :OWN]], 1)
        m = dict(shared)
        m["x_r"] = np.ascontiguousarray(
            stack.reshape(12, 128, N).transpose(1, 0, 2)).astype(F8)
        m["x_own"] = np.ascontiguousarray(
            stack[:, 0:OWN].reshape(12, 128, OWN).transpose(1, 0, 2))
        in_maps.append(m)

    res = run_bass_kernel_spmd(nc, in_maps, list(range(NCORES)))
    out = np.empty((B, N, C, 2), np.float32)
    for c in range(NCORES):
        b, half = divmod(c, 2)
        o = res.results[c]["out_fm"]                 # [12, 128, OWN]
        sl = slice(half * OWN, half * OWN + OWN)
        out[b, sl, :, 0] = o[0:6].reshape(768, OWN).T
        out[b, sl, :, 1] = o[6:12].reshape(768, OWN).T
    if debug:
        return out, res
    return out


# revision 15
# speedup vs baseline: 1.1836x; 1.1836x over previous
"""Complex transformer block (LN->attn->LN->MLP, complex arithmetic) on 8 TRN2 cores.

Sharding: core c handles (batch b = c//2, sequence half = c%2). No collectives:
each core computes K/V over the full 1024-token sequence of its batch (the only
duplicated work) and queries/MLP over its own 512 tokens.

Layout: activations are feature-major [feature partition-blocks, tokens].
Complex tensors are realified as separate real/imag feature planes.

Attention path runs in fp8e4m3 with DoubleRow matmuls (2 K-planes per pass):
x, qkv/v/proj weights and the attention output are fp8; softmax scores/exp
stay bf16/f32. LayerNorm1 is folded into the qkv weights via per-token stat
rows (mu_r, mu_i, std appended to the contraction) with the rstd applied at
PSUM eviction - this keeps the LN off the critical path. The MLP runs in bf16
(fp8 there fails the error budget): LayerNorm2 is materialized once (xh2) and
gelu reads PSUM directly with a fused per-feature bias. Attention scores are
computed transposed ([t2, t1]) so softmax sums reduce via ones-matmuls, and V
is produced pre-transposed by swapping matmul operands. All weights are stored
host-side in the exact SBUF layout so every weight DMA is fully contiguous.
"""
import sys
sys.path.insert(0, "/opt/trn_rl_repo")

from contextlib import ExitStack

import ml_dtypes
import numpy as np

import concourse.bacc as bacc
import concourse.bass as bass
import concourse.mybir as mybir
import concourse.tile as tile
from concourse.bass_utils import run_bass_kernel_spmd

# Prefer the table set that covers the whole softmax chain (square+ln+exp)
# so the greedy act-table-load pass doesn't thrash sets on every block.
_orig_get_tables = bacc.get_activation_tables


def _reordered_tables(arch):
    t = _orig_get_tables(arch)
    keep = {"natural_log_exp_and_others", "gelu_and_others"}
    return {k: (v if k in keep else set()) for k, v in t.items()}


bacc.get_activation_tables = _reordered_tables

dt = mybir.dt
AF = mybir.ActivationFunctionType
ALU = mybir.AluOpType
DR = mybir.MatmulPerfMode.DoubleRow
BF16 = ml_dtypes.bfloat16
F8 = ml_dtypes.float8_e4m3

B, N, C, H, DH, HID = 4, 1024, 768, 12, 64, 3072
NCORES = 8
OWN = 512          # tokens per core
SCALE = DH ** -0.5
EPS = 1e-5


def round_fp32r(x):
    b = np.ascontiguousarray(x, dtype=np.float32).view(np.uint32)
    lsb = (b >> np.uint32(12)) & np.uint32(1)
    return ((b + np.uint32(0x7FF) + lsb) & np.uint32(0xFFFFF000)).view(np.float32)


# --------------------------------------------------------------------------
# device program
# --------------------------------------------------------------------------

def build_nc(debug=False):
    nc = bacc.Bacc(trn_type="TRN2", target_bir_lowering=False)
    f32 = dt.float32
    f32r = dt.float32r
    bf16 = dt.bfloat16
    f8 = dt.float8e4

    # ---- DRAM I/O ----
    x_r = nc.dram_tensor("x_r", [128, 12, N], f8, kind="ExternalInput")
    x_own = nc.dram_tensor("x_own", [128, 12, OWN], f32, kind="ExternalInput")
    w_qkv = nc.dram_tensor("w_qkv", [H, 128, 12, 384], f8, kind="ExternalInput")
    w_qkv_s = nc.dram_tensor("w_qkv_s", [H, 4, 384], f32r, kind="ExternalInput")
    w_v = nc.dram_tensor("w_v", [6, 128, 12, 256], f8, kind="ExternalInput")
    w_v_s = nc.dram_tensor("w_v_s", [4, 1536], f32r, kind="ExternalInput")
    w_proj = nc.dram_tensor("w_proj", [12, 128, 12, 128], f8, kind="ExternalInput")
    w_pb = nc.dram_tensor("w_pb", [128, 12], f32, kind="ExternalInput")
    w_fc1r = nc.dram_tensor("w_fc1r", [24, 128, 6, 128], bf16, kind="ExternalInput")
    w_fc1i = nc.dram_tensor("w_fc1i", [24, 128, 6, 128], bf16, kind="ExternalInput")
    w_fc1in = nc.dram_tensor("w_fc1in", [24, 128, 6, 128], bf16, kind="ExternalInput")
    w_fc1b = nc.dram_tensor("w_fc1b", [128, 24, 2], f32, kind="ExternalInput")
    w_fc2r = nc.dram_tensor("w_fc2r", [6, 128, 24, 128], bf16, kind="ExternalInput")
    w_fc2i = nc.dram_tensor("w_fc2i", [6, 128, 24, 128], bf16, kind="ExternalInput")
    w_fc2in = nc.dram_tensor("w_fc2in", [6, 128, 24, 128], bf16, kind="ExternalInput")
    w_fc2_s = nc.dram_tensor("w_fc2_s", [6, 4, 256], f32r, kind="ExternalInput")
    ones_col = nc.dram_tensor("ones_col", [128, 1], bf16, kind="ExternalInput")
    ones_ab8 = nc.dram_tensor("ones_ab8", [128, 4], f8, kind="ExternalInput")
    ones_s8 = nc.dram_tensor("ones_s8", [128, 1], f8, kind="ExternalInput")
    ones_ab = nc.dram_tensor("ones_ab", [128, 4], bf16, kind="ExternalInput")
    ones_s = nc.dram_tensor("ones_s", [128, 1], bf16, kind="ExternalInput")
    stat_one = nc.dram_tensor("stat_one", [4, OWN], f32r, kind="ExternalInput")
    ident8 = nc.dram_tensor("ident8", [8, 8], f32r, kind="ExternalInput")

    out_fm = nc.dram_tensor("out_fm", [12, 128, OWN], f32, kind="ExternalOutput")

    with tile.TileContext(nc) as tc, ExitStack() as top:
        consts = top.enter_context(tc.tile_pool(name="consts", bufs=1))
        t_ones_col = consts.tile([128, 1], bf16)
        t_ones_ab8 = consts.tile([128, 4], f8)
        t_ones_s8 = consts.tile([128, 1], f8)
        t_ones_ab = consts.tile([128, 4], bf16)
        t_ones_s = consts.tile([128, 1], bf16)
        t_stat_one = consts.tile([4, OWN], f32r)
        t_id8 = consts.tile([8, 8], f32r)
        t_pb = consts.tile([128, 12], f32)
        t_f1b = consts.tile([128, 24, 2], f32)
        t_eps = consts.tile([1, 1], f32)
        nc.sync.dma_start(t_ones_col[:], ones_col[:])
        nc.sync.dma_start(t_ones_ab8[:], ones_ab8[:])
        nc.sync.dma_start(t_ones_s8[:], ones_s8[:])
        nc.sync.dma_start(t_ones_ab[:], ones_ab[:])
        nc.sync.dma_start(t_ones_s[:], ones_s[:])
        nc.sync.dma_start(t_stat_one[:], stat_one[:])
        nc.sync.dma_start(t_id8[:], ident8[:])
        nc.sync.dma_start(t_pb[:], w_pb[:])
        nc.sync.dma_start(t_f1b[:], w_fc1b[:])
        nc.vector.memset(t_eps[:], EPS)

        poolR1 = top.enter_context(tc.tile_pool(name="poolR1", bufs=1))
        xr1 = poolR1.tile([128, 12, OWN], f32, name="xr1")

        with ExitStack() as es_x:
            poolX = es_x.enter_context(tc.tile_pool(name="poolX", bufs=1))
            x8 = poolX.tile([128, 12, N], f8, name="x8")
            pdram = es_x.enter_context(
                tc.tile_pool(name="pdram", bufs=1, space="DRAM"))
            rstd_dram = pdram.tile([1, N], f32, name="rstd_dram")
            stat1s = [poolX.tile([4, 512], f32r, name=f"stat1_{ch}")
                      for ch in range(2)]
            rstd_bc1s = [poolX.tile([128, 512], f32, name=f"rstd_bc1_{ch}")
                         for ch in range(2)]
            rstdT = poolX.tile([128, 8], f32, name="rstdT")
            for kb in range(12):
                nc.sync.dma_start(x8[:, kb, :], x_r[:, kb, :])

            # ---------------- phase A: LN1 stats over full sequence --------
            with ExitStack() as es_a:
                pa = es_a.enter_context(tc.tile_pool(name="pa_sb", bufs=3))
                pa_ps = es_a.enter_context(
                    tc.tile_pool(name="pa_ps", bufs=2, space="PSUM"))
                pa_sc = es_a.enter_context(tc.tile_pool(name="pa_sc", bufs=2))
                mu_pss = [pa_ps.tile([2, 512], f32, tag=f"mu{ch}",
                                     name=f"mu{ch}", bufs=1) for ch in range(2)]
                s_pss = [pa_ps.tile([1, 512], f32, tag=f"s{ch}",
                                    name=f"s{ch}", bufs=1) for ch in range(2)]
                for kb in range(12):
                    sq = pa.tile([128, N], f8, tag="sq", name=f"sq{kb}")
                    nc.scalar.activation(sq[:], x8[:, kb, :], AF.Square)
                    lhs = t_ones_ab8[:, 0:2] if kb < 6 else t_ones_ab8[:, 2:4]
                    for ch in range(2):
                        sl = slice(ch * 512, ch * 512 + 512)
                        nc.tensor.matmul(mu_pss[ch][:], lhs, x8[:, kb, sl],
                                         start=(kb == 0), stop=(kb == 11))
                        nc.tensor.matmul(s_pss[ch][:], t_ones_s8[:], sq[:, sl],
                                         start=(kb == 0), stop=(kb == 11))
                for ch in range(2):
                    sl = slice(ch * 512, ch * 512 + 512)
                    mu_ps = mu_pss[ch]
                    s_ps = s_pss[ch]
                    # var = S - mu_r^2 - mu_i^2 ; std = exp(.5 ln(var+eps))
                    mu_sb = pa_sc.tile([2, 512], f32, tag="musb", name=f"musb{ch}")
                    mu_fl = pa_sc.tile([1, 2, 512], f32, tag="mufl", name=f"mufl{ch}")
                    var = pa_sc.tile([1, 512], f32, tag="var", name=f"var{ch}")
                    lnv = pa_sc.tile([1, 512], f32, tag="lnv", name=f"lnv{ch}")
                    s_c = pa_sc.tile([1, 512], f32, tag="sc_", name=f"sc_{ch}")
                    nc.vector.tensor_scalar(mu_sb[:], mu_ps[:], 1.0 / C, None,
                                            op0=ALU.mult)
                    nc.vector.tensor_scalar(s_c[:], s_ps[:], 1.0 / C, None,
                                            op0=ALU.mult)
                    nc.sync.dma_start(mu_fl[:, 0, :], mu_sb[0:1, :])
                    nc.sync.dma_start(mu_fl[:, 1, :], mu_sb[1:2, :])
                    sq_mu = pa_sc.tile([1, 2, 512], f32, tag="sqmu", name=f"sqmu{ch}")
                    nc.vector.tensor_tensor(sq_mu[:], mu_fl[:], mu_fl[:],
                                            op=ALU.mult)
                    nc.vector.tensor_tensor(var[:], s_c[:], sq_mu[:, 0, :],
                                            op=ALU.subtract)
                    nc.vector.tensor_tensor(var[:], var[:], sq_mu[:, 1, :],
                                            op=ALU.subtract)
                    nc.scalar.activation(lnv[:], var[:], AF.Ln, bias=t_eps[:])
                    # stats rows: 0=mu_r 1=mu_i 2=std
                    nc.vector.tensor_copy(stat1s[ch][0:2, :], mu_sb[:])
                    std_row = pa_sc.tile([1, 512], f32r, tag="stdr", name=f"stdr{ch}")
                    nc.scalar.activation(std_row[:], lnv[:], AF.Exp, scale=0.5)
                    nc.sync.dma_start(stat1s[ch][2:3, :], std_row[:])
                    rstd_row = pa_sc.tile([1, 512], f32r, tag="rst", name=f"rst{ch}")
                    nc.scalar.activation(rstd_row[:], lnv[:], AF.Exp, scale=-0.5)
                    nc.sync.dma_start(rstd_dram[:, sl], rstd_row[:].bitcast(f32))
                    nc.gpsimd.partition_broadcast(
                        rstd_bc1s[ch][:], rstd_row[:].bitcast(f32))
                # rstd transposed: rstdT[p, t2b] = rstd[t2b*128 + p]
                rstd8 = pa_sc.tile([8, 128], f32, tag="r8", name="rstd8")
                nc.sync.dma_start(
                    rstd8[:], rstd_dram[:].rearrange("o (a b) -> (o a) b", a=8))
                rstdT_ps = pa_ps.tile([128, 8], f32, tag="rtps", name="rtps")
                nc.tensor.transpose(rstdT_ps[:], rstd8[:], t_id8[:].bitcast(f32))
                nc.vector.tensor_copy(rstdT[:], rstdT_ps[:])

            # ---------------- phase BC: qkv + attention per head ----------
            es_attn = ExitStack()
            attnp = es_attn.enter_context(tc.tile_pool(name="attnp", bufs=1))
            attn = attnp.tile([128, 12, OWN], f8, name="attn")
            es_b = ExitStack()
            pq = es_b.enter_context(tc.tile_pool(name="pq", bufs=1))
            pk = es_b.enter_context(tc.tile_pool(name="pk", bufs=1))
            pvt = es_b.enter_context(tc.tile_pool(name="pvt", bufs=2))
            pwv = es_b.enter_context(tc.tile_pool(name="pwv", bufs=1))
            pwq = es_b.enter_context(tc.tile_pool(name="pwq", bufs=2))
            pet = es_b.enter_context(tc.tile_pool(name="pet", bufs=5))
            psc = es_b.enter_context(tc.tile_pool(name="psc", bufs=4))
            prd = es_b.enter_context(tc.tile_pool(name="prd", bufs=2))
            ps_rot = es_b.enter_context(
                tc.tile_pool(name="ps_rot", bufs=2, space="PSUM"))
            ps_sc = es_b.enter_context(
                tc.tile_pool(name="ps_sc", bufs=2, space="PSUM"))
            ps_acc = es_b.enter_context(
                tc.tile_pool(name="ps_acc", bufs=2, space="PSUM"))
            pdram_rd = es_b.enter_context(
                tc.tile_pool(name="pdram_rd", bufs=2, space="DRAM"))
            vt_pair = None
            et_fifo = []
            acc_ps = {}
            LAG = 6

            def emit_avden(ent):
                h2, t2b2, et2, vt2 = ent
                slot2 = h2 % 2
                if t2b2 == 0:
                    acc_ps[h2] = (
                        ps_acc.tile([128, OWN], f32, tag="av", name=f"av{h2}",
                                    bufs=1),
                        ps_acc.tile([1, OWN], f32, tag="den", name=f"den{h2}",
                                    bufs=1),
                    )
                av2, den2 = acc_ps[h2]
                nc.tensor.matmul(den2[:], t_ones_col[:], et2,
                                 start=(t2b2 == 0), stop=(t2b2 == 7))
                dsl2 = slice(slot2 * 128, slot2 * 128 + 128)
                nc.tensor.matmul(av2[:], vt2[:, t2b2, dsl2], et2,
                                 start=(t2b2 == 0), stop=(t2b2 == 7))
                if t2b2 == 7:
                    den_sb = prd.tile([1, OWN], f32, tag="den_sb",
                                      name=f"dsb{h2}", bufs=1)
                    nc.vector.tensor_copy(den_sb[:], den2[:])
                    den_dram = pdram_rd.tile([1, OWN], f32, tag="dend",
                                             name=f"dend{h2}")
                    nc.sync.dma_start(den_dram[:], den_sb[:])
                    den_sp = prd.tile([128, 4], f32, tag="den_sp",
                                      name=f"dsp{h2}", bufs=1)
                    nc.sync.dma_start(
                        den_sp[:],
                        den_dram[:].rearrange("o (a b) -> (o a) b", a=128))
                    rd_sp = prd.tile([128, 4], f32, tag="rd_sp",
                                     name=f"rsp{h2}", bufs=1)
                    nc.vector.reciprocal(rd_sp[:], den_sp[:])
                    rd_dram = pdram_rd.tile([1, OWN], f32, tag="rdd",
                                            name=f"rdd{h2}")
                    nc.sync.dma_start(
                        rd_dram[:].rearrange("o (a b) -> (o a) b", a=128),
                        rd_sp[:])
                    rd_bc = prd.tile([128, OWN], f32, tag="rd_bc",
                                     name=f"rdbc{h2}", bufs=1)
                    rd_bcast_ap = bass.AP(tensor=rd_dram.tensor,
                                          offset=rd_dram[:].offset,
                                          ap=[[0, 128]] + rd_dram[:].ap[1:])
                    nc.sync.dma_start(rd_bc[:], rd_bcast_ap)
                    nc.vector.tensor_tensor(attn[:, h2, :], av2[:], rd_bc[:],
                                            op=ALU.mult)
                    del acc_ps[h2]

            for h in range(H):
                pair, slot = divmod(h, 2)
                # qkv for head h: q1=[q_r;-q_i], q3=[q_i;q_r], k=[k_r;k_i]
                q_t = pq.tile([128, 2, OWN], bf16, tag="q", name=f"q{h}")
                k_t = pk.tile([128, N], bf16, tag="k", name=f"k{h}")
                wqkv_t = pwq.tile([128, 12, 384], f8, tag="wqkv",
                                  name=f"wqkv{h}")
                wqs_t = pwq.tile([4, 384], f32r, tag="wqs", name=f"wqs{h}")
                nc.sync.dma_start(wqkv_t[:], w_qkv[h])
                nc.sync.dma_start(wqs_t[:], w_qkv_s[h])
                q1_ps = ps_rot.tile([128, OWN], f32, tag="rot", name=f"q1ps{h}")
                q3_ps = ps_rot.tile([128, OWN], f32, tag="rot", name=f"q3ps{h}")
                for p in range(6):
                    kp = slice(2 * p, 2 * p + 2)
                    st = (p == 0)
                    nc.tensor.matmul(q1_ps[:], wqkv_t[:, kp, 0:128],
                                     x8[:, kp, 0:OWN], start=st, stop=False,
                                     perf_mode=DR)
                    nc.tensor.matmul(q3_ps[:], wqkv_t[:, kp, 128:256],
                                     x8[:, kp, 0:OWN], start=st, stop=False,
                                     perf_mode=DR)
                nc.tensor.matmul(q1_ps[:], wqs_t[:, 0:128], stat1s[0][:],
                                 start=False, stop=True)
                nc.tensor.matmul(q3_ps[:], wqs_t[:, 128:256], stat1s[0][:],
                                 start=False, stop=True)
                nc.vector.tensor_tensor(q_t[:, 0, :], q1_ps[:],
                                        rstd_bc1s[0][:], op=ALU.mult)
                nc.vector.tensor_tensor(q_t[:, 1, :], q3_ps[:],
                                        rstd_bc1s[0][:], op=ALU.mult)
                k0_ps = ps_rot.tile([128, 512], f32, tag="rot", name=f"k0ps{h}")
                k1_ps = ps_rot.tile([128, 512], f32, tag="rot", name=f"k1ps{h}")
                for p in range(6):
                    kp = slice(2 * p, 2 * p + 2)
                    st = (p == 0)
                    nc.tensor.matmul(k0_ps[:], wqkv_t[:, kp, 256:384],
                                     x8[:, kp, 0:512], start=st, stop=False,
                                     perf_mode=DR)
                    nc.tensor.matmul(k1_ps[:], wqkv_t[:, kp, 256:384],
                                     x8[:, kp, 512:N], start=st, stop=False,
                                     perf_mode=DR)
                nc.tensor.matmul(k0_ps[:], wqs_t[:, 256:384], stat1s[0][:],
                                 start=False, stop=True)
                nc.tensor.matmul(k1_ps[:], wqs_t[:, 256:384], stat1s[1][:],
                                 start=False, stop=True)
                nc.vector.tensor_tensor(k_t[:, 0:512], k0_ps[:],
                                        rstd_bc1s[0][:], op=ALU.mult)
                nc.vector.tensor_tensor(k_t[:, 512:N], k1_ps[:],
                                        rstd_bc1s[1][:], op=ALU.mult)
                if slot == 0:
                    # V^T for this head pair: [t2, d] via swapped operands
                    wv_t = pwv.tile([128, 12, 256], f8, tag="wv",
                                    name=f"wv{pair}")
                    wv_s = pwv.tile([4, 256], f32r, tag="wvs",
                                    name=f"wvs{pair}")
                    csl = slice(pair * 256, pair * 256 + 256)
                    nc.sync.dma_start(wv_t[:], w_v[pair])
                    nc.sync.dma_start(wv_s[:], w_v_s[:, csl])
                    vt_pair = pvt.tile([128, 8, 256], bf16, tag="vt",
                                       name=f"vt{pair}")
                    for t2b in range(8):
                        t2s = slice(t2b * 128, t2b * 128 + 128)
                        vt_ps = ps_rot.tile([128, 256], f32, tag="rot",
                                            name=f"vtps{pair}_{t2b}")
                        for p in range(6):
                            kp = slice(2 * p, 2 * p + 2)
                            nc.tensor.matmul(vt_ps[:], x8[:, kp, t2s],
                                             wv_t[:, kp, :],
                                             start=(p == 0), stop=False,
                                             perf_mode=DR)
                        st1 = stat1s[t2b // 4]
                        t2l = slice((t2b % 4) * 128, (t2b % 4) * 128 + 128)
                        nc.tensor.matmul(vt_ps[:], st1[:, t2l], wv_s[:],
                                         start=False, stop=True)
                        nc.vector.tensor_scalar(
                            vt_pair[:, t2b, :], vt_ps[:],
                            rstdT[:, t2b:t2b + 1], None, op0=ALU.mult)
                # scores + exp chain, batched over block pairs;
                # den/av matmuls lag by LAG sub-blocks
                for t2p in range(4):
                    t2s0 = slice(t2p * 256, t2p * 256 + 128)
                    t2s1 = slice(t2p * 256 + 128, t2p * 256 + 256)
                    sr_pair = ps_sc.tile([128, 2, OWN], f32, tag="scp",
                                         name=f"srp{h}_{t2p}")
                    si_pair = ps_sc.tile([128, 2, OWN], f32, tag="scp",
                                         name=f"sip{h}_{t2p}")
                    nc.tensor.matmul(sr_pair[:, 0, :], k_t[:, t2s0],
                                     q_t[:, 0, :], start=True, stop=True)
                    nc.tensor.matmul(si_pair[:, 0, :], k_t[:, t2s0],
                                     q_t[:, 1, :], start=True, stop=True)
                    nc.tensor.matmul(sr_pair[:, 1, :], k_t[:, t2s1],
                                     q_t[:, 0, :], start=True, stop=True)
                    nc.tensor.matmul(si_pair[:, 1, :], k_t[:, t2s1],
                                     q_t[:, 1, :], start=True, stop=True)
                    sqr = psc.tile([128, 2, OWN], f32, tag="sqr",
                                   name=f"sqr{h}_{t2p}")
                    sqi = psc.tile([128, 2, OWN], f32, tag="sqi",
                                   name=f"sqi{h}_{t2p}")
                    nc.scalar.activation(sqr[:], sr_pair[:], AF.Square)
                    nc.scalar.activation(sqi[:], si_pair[:], AF.Square)
                    # in-place chain on sqr: m2 -> ln -> 0.5ln -> mag -> exp
                    nc.vector.tensor_tensor(sqr[:], sqr[:], sqi[:],
                                            op=ALU.add)
                    nc.scalar.activation(sqr[:], sqr[:], AF.Ln)
                    nc.scalar.activation(sqr[:], sqr[:], AF.Exp, scale=0.5)
                    et = pet.tile([128, 2, OWN], bf16, tag="et",
                                  name=f"et{h}_{t2p}")
                    nc.scalar.activation(et[:], sqr[:], AF.Exp)
                    for sub in range(2):
                        et_fifo.append((h, t2p * 2 + sub, et[:, sub, :],
                                        vt_pair))
                        while len(et_fifo) > LAG:
                            emit_avden(et_fifo.pop(0))
            for ent in et_fifo:
                emit_avden(ent)
            et_fifo.clear()
            es_b.close()

            # ------------- phase D: proj + residual --------------------
            nc.sync.dma_start(xr1[:], x_own[:])
            for opb in range(12):
                nc.vector.tensor_scalar(xr1[:, opb, :], xr1[:, opb, :],
                                        t_pb[:, opb:opb + 1], None,
                                        op0=ALU.add)
            r1r = poolR1.tile([128, 12, OWN], bf16, name="r1r")
            with ExitStack() as es_d:
                pwp = es_d.enter_context(tc.tile_pool(name="pwp", bufs=3))
                ps_d = es_d.enter_context(
                    tc.tile_pool(name="ps_d", bufs=4, space="PSUM"))
                for opb in range(12):
                    wp_t = pwp.tile([128, 12, 128], f8, tag="wp",
                                    name=f"wp{opb}")
                    nc.sync.dma_start(wp_t[:], w_proj[opb])
                    pr_ps = ps_d.tile([128, OWN], f32, tag="pr",
                                      name=f"prps{opb}")
                    for p in range(6):
                        kp = slice(2 * p, 2 * p + 2)
                        nc.tensor.matmul(pr_ps[:], wp_t[:, kp, :],
                                         attn[:, kp, :],
                                         start=(p == 0), stop=(p == 5),
                                         perf_mode=DR)
                    nc.vector.tensor_tensor(xr1[:, opb, :], pr_ps[:],
                                            xr1[:, opb, :], op=ALU.add)
                    nc.vector.tensor_copy(r1r[:, opb, :], xr1[:, opb, :])
            es_attn.close()

        # ---------------- phase E: LN2 stats + normalized r1 --------------
        poolE = top.enter_context(tc.tile_pool(name="poolE", bufs=1))
        xh2 = poolE.tile([128, 12, OWN], bf16, name="xh2")
        with ExitStack() as es_e:
            pe = es_e.enter_context(tc.tile_pool(name="pe_sb", bufs=1))
            pdram2 = es_e.enter_context(
                tc.tile_pool(name="pdram2", bufs=1, space="DRAM"))
            pe_ps = es_e.enter_context(
                tc.tile_pool(name="pe_ps", bufs=2, space="PSUM"))
            sq2s = []
            for kb in range(12):
                sq2 = pe.tile([128, OWN], bf16, tag="sq2", name=f"sq2_{kb}", bufs=12)
                nc.scalar.activation(sq2[:], r1r[:, kb, :], AF.Square)
                sq2s.append(sq2)
            mu2_ps = pe_ps.tile([2, OWN], f32, tag="mu2", name="mu2")
            s2_ps = pe_ps.tile([1, OWN], f32, tag="s2", name="s2")
            for kb in range(12):
                lhs = t_ones_ab[:, 0:2] if kb < 6 else t_ones_ab[:, 2:4]
                nc.tensor.matmul(mu2_ps[:], lhs, r1r[:, kb, :],
                                 start=(kb == 0), stop=(kb == 11))
                nc.tensor.matmul(s2_ps[:], t_ones_s[:], sq2s[kb][:],
                                 start=(kb == 0), stop=(kb == 11))
            mu2_sb = pe.tile([2, OWN], f32, tag="emusb", name="emusb")
            mu2_fl = pe.tile([1, 2, OWN], f32, tag="emufl", name="emufl")
            var = pe.tile([1, OWN], f32, tag="evar", name="evar")
            lnv = pe.tile([1, OWN], f32, tag="elnv", name="elnv")
            s2_c = pe.tile([1, OWN], f32, tag="es2c", name="es2c")
            nc.vector.tensor_scalar(mu2_sb[:], mu2_ps[:], 1.0 / C, None,
                                    op0=ALU.mult)
            nc.vector.tensor_scalar(s2_c[:], s2_ps[:], 1.0 / C, None,
                                    op0=ALU.mult)
            nc.sync.dma_start(mu2_fl[:, 0, :], mu2_sb[0:1, :])
            nc.sync.dma_start(mu2_fl[:, 1, :], mu2_sb[1:2, :])
            sq_mu2 = pe.tile([1, 2, OWN], f32, tag="esqmu", name="esqmu")
            nc.vector.tensor_tensor(sq_mu2[:], mu2_fl[:], mu2_fl[:], op=ALU.mult)
            nc.vector.tensor_tensor(var[:], s2_c[:], sq_mu2[:, 0, :],
                                    op=ALU.subtract)
            nc.vector.tensor_tensor(var[:], var[:], sq_mu2[:, 1, :],
                                    op=ALU.subtract)
            nc.scalar.activation(lnv[:], var[:], AF.Ln, bias=t_eps[:])
            rstd2_row = pe.tile([1, OWN], f32, tag="ers", name="ers")
            nc.scalar.activation(rstd2_row[:], lnv[:], AF.Exp, scale=-0.5)
            mu2r_bc = pe.tile([128, OWN], f32, tag="m2rbc", name="m2rbc")
            mu2i_bc = pe.tile([128, OWN], f32, tag="m2ibc", name="m2ibc")
            rstd2_bc = pe.tile([128, OWN], f32, tag="r2bc", name="r2bc")
            nc.gpsimd.partition_broadcast(mu2r_bc[:], mu2_fl[:, 0, :])
            nc.gpsimd.partition_broadcast(mu2i_bc[:], mu2_fl[:, 1, :])
            nc.gpsimd.partition_broadcast(rstd2_bc[:], rstd2_row[:])
            for kb in range(12):
                mbc = mu2r_bc if kb < 6 else mu2i_bc
                nc.vector.tensor_tensor(xh2[:, kb, :], r1r[:, kb, :],
                                        mbc[:], op=ALU.subtract)
                nc.vector.tensor_tensor(xh2[:, kb, :], xh2[:, kb, :],
                                        rstd2_bc[:], op=ALU.mult)

        # ---------------- phase F: MLP, single 512-token pass -------------
        with ExitStack() as es_f:
            ph = es_f.enter_context(tc.tile_pool(name="ph", bufs=1))
            pw1 = es_f.enter_context(tc.tile_pool(name="pw1", bufs=6))
            pw2 = es_f.enter_context(tc.tile_pool(name="pw2", bufs=4))
            pout = es_f.enter_context(tc.tile_pool(name="pout", bufs=2))
            ps_f = es_f.enter_context(
                tc.tile_pool(name="ps_f", bufs=4, space="PSUM"))
            h_t = ph.tile([128, 48, OWN], bf16, name="h_t")
            for Cb in range(24):
                w1r_t = pw1.tile([128, 6, 128], bf16, tag="w1r",
                                 name=f"w1r{Cb}")
                w1i_t = pw1.tile([128, 6, 128], bf16, tag="w1i",
                                 name=f"w1i{Cb}")
                w1in_t = pw1.tile([128, 6, 128], bf16, tag="w1in",
                                  name=f"w1in{Cb}")
                nc.sync.dma_start(w1r_t[:], w_fc1r[Cb])
                nc.sync.dma_start(w1i_t[:], w_fc1i[Cb])
                nc.sync.dma_start(w1in_t[:], w_fc1in[Cb])
                hr_ps = ps_f.tile([128, OWN], f32, tag="fps",
                                  name=f"hrps{Cb}")
                hi_ps = ps_f.tile([128, OWN], f32, tag="fps",
                                  name=f"hips{Cb}")
                for kb in range(6):
                    st = (kb == 0)
                    nc.tensor.matmul(hr_ps[:], w1r_t[:, kb, :],
                                     xh2[:, kb, :], start=st, stop=False)
                    nc.tensor.matmul(hi_ps[:], w1i_t[:, kb, :],
                                     xh2[:, kb, :], start=st, stop=False)
                for kb in range(6):
                    lst = (kb == 5)
                    nc.tensor.matmul(hr_ps[:], w1in_t[:, kb, :],
                                     xh2[:, 6 + kb, :], start=False,
                                     stop=lst)
                    nc.tensor.matmul(hi_ps[:], w1r_t[:, kb, :],
                                     xh2[:, 6 + kb, :], start=False,
                                     stop=lst)
                nc.scalar.activation(h_t[:, Cb, :], hr_ps[:], AF.Gelu,
                                     bias=t_f1b[:, Cb, 0:1])
                nc.scalar.activation(h_t[:, 24 + Cb, :], hi_ps[:], AF.Gelu,
                                     bias=t_f1b[:, Cb, 1:2])
            for j in range(6):
                w2r_t = pw2.tile([128, 24, 128], bf16, tag="w2r",
                                 name=f"w2r{j}")
                w2i_t = pw2.tile([128, 24, 128], bf16, tag="w2i",
                                 name=f"w2i{j}")
                w2in_t = pw2.tile([128, 24, 128], bf16, tag="w2in",
                                  name=f"w2in{j}")
                w2s_t = pw2.tile([4, 256], f32r, tag="w2s",
                                 name=f"w2s{j}")
                nc.sync.dma_start(w2r_t[:], w_fc2r[j])
                nc.sync.dma_start(w2i_t[:], w_fc2i[j])
                nc.sync.dma_start(w2in_t[:], w_fc2in[j])
                nc.sync.dma_start(w2s_t[:], w_fc2_s[j])
                or_ps = ps_f.tile([128, OWN], f32, tag="fps",
                                  name=f"orps{j}")
                oi_ps = ps_f.tile([128, OWN], f32, tag="fps",
                                  name=f"oips{j}")
                for kb in range(24):
                    st = (kb == 0)
                    nc.tensor.matmul(or_ps[:], w2r_t[:, kb, :], h_t[:, kb, :],
                                     start=st, stop=False)
                    nc.tensor.matmul(oi_ps[:], w2i_t[:, kb, :], h_t[:, kb, :],
                                     start=st, stop=False)
                for kb in range(24):
                    nc.tensor.matmul(or_ps[:], w2in_t[:, kb, :],
                                     h_t[:, 24 + kb, :],
                                     start=False, stop=False)
                    nc.tensor.matmul(oi_ps[:], w2r_t[:, kb, :],
                                     h_t[:, 24 + kb, :],
                                     start=False, stop=False)
                nc.tensor.matmul(or_ps[:], w2s_t[:, 0:128],
                                 t_stat_one[:], start=False, stop=True)
                nc.tensor.matmul(oi_ps[:], w2s_t[:, 128:256],
                                 t_stat_one[:], start=False, stop=True)
                o_r = pout.tile([128, OWN], f32, tag="o", name=f"or{j}")
                o_i = pout.tile([128, OWN], f32, tag="o", name=f"oi{j}")
                nc.vector.tensor_tensor(o_r[:], or_ps[:], xr1[:, j, :],
                                        op=ALU.add)
                nc.vector.tensor_tensor(o_i[:], oi_ps[:], xr1[:, 6 + j, :],
                                        op=ALU.add)
                nc.sync.dma_start(out_fm[j], o_r[:])
                nc.sync.dma_start(out_fm[6 + j], o_i[:])
    nc.compile()
    return nc


# --------------------------------------------------------------------------
# host side
# --------------------------------------------------------------------------

def _cx(a):
    return a[..., 0].astype(np.float64) + 1j * a[..., 1].astype(np.float64)


def _kcols(Wp, wsum, wb, plane, scale=1.0):
    """K-profile [1539, m] for output features with complex weight rows Wp
    [m, 768], LN fold sums wsum [m], bias-column wb [m]. K rows: xr(768),
    xi(768), mu_r, mu_i, std."""
    m = Wp.shape[0]
    out = np.zeros((1539, m), np.float64)
    if plane == "r":
        out[0:768] = Wp.real.T
        out[768:1536] = -Wp.imag.T
        out[1536] = -wsum.real
        out[1537] = wsum.imag
        out[1538] = wb.real
    else:
        out[0:768] = Wp.imag.T
        out[768:1536] = Wp.real.T
        out[1536] = -wsum.imag
        out[1537] = -wsum.real
        out[1538] = wb.imag
    return out * scale


def _bf(a):
    return np.ascontiguousarray(a).astype(BF16)


def _f8(a):
    return np.ascontiguousarray(a).astype(F8)


def _pmajor(a):
    """[kb, 128, n] -> [128, kb, n] partition-major contiguous."""
    return np.ascontiguousarray(np.transpose(a, (1, 0, 2)))


def _prep_weights(inputs):
    n1 = _cx(inputs["n1_w"]); b1 = _cx(inputs["n1_b"])
    n2 = _cx(inputs["n2_w"]); b2 = _cx(inputs["n2_b"])
    Wqkv = _cx(inputs["qkv_w"])          # [2304, 768]
    Wp = _cx(inputs["proj_w"])           # [768, 768]
    bp = _cx(inputs["proj_b"])           # [768]
    W1 = _cx(inputs["fc1_w"])            # [3072, 768]
    bf1 = _cx(inputs["fc1_b"])           # [3072]
    W2 = _cx(inputs["fc2_w"])            # [768, 3072]
    bf2 = _cx(inputs["fc2_b"])           # [768]

    d = {}
    # ---- qkv (LN1-folded) ----
    Wq, Wk, Wv = Wqkv[0:768], Wqkv[768:1536], Wqkv[1536:2304]

    def fold1(W):
        Wf = W * n1[None, :]
        return Wf, Wf.sum(1), W @ b1

    w_qkv = np.zeros((H, 128, 12, 384), F8)
    w_qkv_s = np.zeros((H, 4, 384), np.float32)
    for h in range(H):
        rows = slice(h * DH, (h + 1) * DH)
        Qf, Qs, Qb = fold1(Wq[rows])
        Kf, Ks, Kb_ = fold1(Wk[rows])
        q1 = np.hstack([_kcols(Qf, Qs, Qb, "r", SCALE),
                        _kcols(Qf, Qs, Qb, "i", -SCALE)])
        q3 = np.hstack([_kcols(Qf, Qs, Qb, "i", SCALE),
                        _kcols(Qf, Qs, Qb, "r", SCALE)])
        kk = np.hstack([_kcols(Kf, Ks, Kb_, "r"), _kcols(Kf, Ks, Kb_, "i")])
        blk = np.hstack([q1, q3, kk]).astype(np.float32)       # [1539, 384]
        w_qkv[h] = _f8(_pmajor(blk[0:1536].reshape(12, 128, 384)))
        w_qkv_s[h, 0:3] = blk[1536:1539]
    d["w_qkv"] = w_qkv
    d["w_qkv_s"] = round_fp32r(w_qkv_s)

    # ---- v (LN1-folded), rhs layout; cols: pair*256+slot*128+plane*64+dh
    wv_full = np.zeros((1539, 1536), np.float64)
    for h in range(H):
        rows = slice(h * DH, (h + 1) * DH)
        Vf, Vs, Vb = fold1(Wv[rows])
        base = h * 128
        wv_full[:, base:base + 64] = _kcols(Vf, Vs, Vb, "r")
        wv_full[:, base + 64:base + 128] = _kcols(Vf, Vs, Vb, "i")
    w_v = np.zeros((6, 128, 12, 256), F8)
    for pair in range(6):
        csl = slice(pair * 256, pair * 256 + 256)
        w_v[pair] = _f8(_pmajor(wv_full[0:1536, csl].reshape(12, 128, 256)))
    d["w_v"] = w_v
    wvs = np.zeros((4, 1536), np.float32)
    wvs[0:3] = wv_full[1536:1539]
    d["w_v_s"] = round_fp32r(wvs)

    # ---- proj; K rows = attn features: per head [a_r(64); a_i(64)] ----
    w_proj = np.zeros((12, 128, 12, 128), F8)
    w_pb = np.zeros((128, 12), np.float32)
    for opb in range(12):
        plane = "r" if opb < 6 else "i"
        orow = slice((opb % 6) * 128, (opb % 6) * 128 + 128)
        Wpo = Wp[orow]                               # [128, 768] complex
        prof = np.zeros((1536, 128), np.float64)
        for hh in range(H):
            cols = slice(hh * DH, (hh + 1) * DH)
            if plane == "r":
                prof[hh * 128:hh * 128 + 64] = Wpo.real[:, cols].T
                prof[hh * 128 + 64:hh * 128 + 128] = -Wpo.imag[:, cols].T
            else:
                prof[hh * 128:hh * 128 + 64] = Wpo.imag[:, cols].T
                prof[hh * 128 + 64:hh * 128 + 128] = Wpo.real[:, cols].T
        w_proj[opb] = _f8(_pmajor(prof.reshape(12, 128, 128)))
        w_pb[:, opb] = (bp.real if plane == "r" else bp.imag)[orow]
    d["w_proj"] = w_proj
    d["w_pb"] = w_pb

    # ---- fc1 (gain-folded; bias separate; LN2 applied via xh2) ----
    W1f = W1 * n2[None, :]
    W1b = W1 @ b2 + bf1
    w_fc1r = np.zeros((24, 128, 6, 128), BF16)
    w_fc1i = np.zeros((24, 128, 6, 128), BF16)
    w_fc1in = np.zeros((24, 128, 6, 128), BF16)
    w_fc1b = np.zeros((128, 24, 2), np.float32)
    for Cb in range(24):
        orow = slice(Cb * 128, (Cb + 1) * 128)
        tr = np.zeros((6, 128, 128), np.float64)
        ti = np.zeros((6, 128, 128), np.float64)
        for kb in range(6):
            icol = slice(kb * 128, (kb + 1) * 128)
            tr[kb] = W1f.real[orow, icol].T
            ti[kb] = W1f.imag[orow, icol].T
        w_fc1r[Cb] = _bf(_pmajor(tr))
        w_fc1i[Cb] = _bf(_pmajor(ti))
        w_fc1in[Cb] = _bf(_pmajor(-ti))
        w_fc1b[:, Cb, 0] = W1b.real[orow]
        w_fc1b[:, Cb, 1] = W1b.imag[orow]
    d["w_fc1r"] = w_fc1r
    d["w_fc1i"] = w_fc1i
    d["w_fc1in"] = w_fc1in
    d["w_fc1b"] = w_fc1b

    # ---- fc2 (plain + bias) ----
    w_fc2r = np.zeros((6, 128, 24, 128), BF16)
    w_fc2i = np.zeros((6, 128, 24, 128), BF16)
    w_fc2in = np.zeros((6, 128, 24, 128), BF16)
    w_fc2_s = np.zeros((6, 4, 256), np.float32)
    for j in range(6):
        orow = slice(j * 128, (j + 1) * 128)
        tr = np.zeros((24, 128, 128), np.float64)
        ti = np.zeros((24, 128, 128), np.float64)
        for kb in range(24):
            icol = slice(kb * 128, (kb + 1) * 128)
            tr[kb] = W2.real[orow, icol].T
            ti[kb] = W2.imag[orow, icol].T
        w_fc2r[j] = _bf(_pmajor(tr))
        w_fc2i[j] = _bf(_pmajor(ti))
        w_fc2in[j] = _bf(_pmajor(-ti))
        w_fc2_s[j, 0, 0:128] = bf2.real[orow]
        w_fc2_s[j, 0, 128:256] = bf2.imag[orow]
    d["w_fc2r"] = w_fc2r
    d["w_fc2i"] = w_fc2i
    d["w_fc2in"] = w_fc2in
    d["w_fc2_s"] = round_fp32r(w_fc2_s)

    # ---- consts ----
    d["ones_col"] = np.ones((128, 1), BF16)
    oab = np.zeros((128, 4), np.float32)
    oab[:, 0] = 1.0
    oab[:, 3] = 1.0
    d["ones_ab8"] = oab.astype(F8)
    d["ones_s8"] = np.ones((128, 1), F8)
    d["ones_ab"] = oab.astype(BF16)
    d["ones_s"] = np.ones((128, 1), BF16)
    so = np.zeros((4, OWN), np.float32)
    so[0] = 1.0
    d["stat_one"] = so
    d["ident8"] = np.eye(8, dtype=np.float32)
    return d


_NC_CACHE = {}


def kernel(**inputs):
    debug = bool(inputs.pop("_debug", False))
    if debug not in _NC_CACHE:
        _NC_CACHE[debug] = build_nc(debug=debug)
    nc = _NC_CACHE[debug]

    shared = _prep_weights(inputs)
    x = np.asarray(inputs["x"], np.float32)          # [B, N, C, 2]

    in_maps = []
    for c in range(NCORES):
        b, half = divmod(c, 2)
        xr_ = x[b, :, :, 0].T                        # [768, 1024]
        xi_ = x[b, :, :, 1].T
        stack = np.concatenate([xr_, xi_], 0)        # [1536, 1024]
        if half == 1:
            stack = np.concatenate([stack[:, OWN:], stack[:, :OWN]], 1)
        m = dict(shared)
        m["x_r"] = np.ascontiguousarray(
            stack.reshape(12, 128, N).transpose(1, 0, 2)).astype(F8)
        m["x_own"] = np.ascontiguousarray(
            stack[:, 0:OWN].reshape(12, 128, OWN).transpose(1, 0, 2))
        in_maps.append(m)

    res = run_bass_kernel_spmd(nc, in_maps, list(range(NCORES)))
    out = np.empty((B, N, C, 2), np.float32)
    for c in range(NCORES):
        b, half = divmod(c, 2)
        o = res.results[c]["out_fm"]                 # [12, 128, OWN]
        sl = slice(half * OWN, half * OWN + OWN)
        out[b, sl, :, 0] = o[0:6].reshape(768, OWN).T
        out[b, sl, :, 1] = o[6:12].reshape(768, OWN).T
    if debug:
        return out, res
    return out


# revision 16
# speedup vs baseline: 1.2909x; 1.0906x over previous
"""Complex transformer block (LN->attn->LN->MLP, complex arithmetic) on 8 TRN2 cores.

Sharding: core c handles (batch b = c//2, sequence half = c%2). No collectives:
each core computes K/V over the full 1024-token sequence of its batch (the only
duplicated work) and queries/MLP over its own 512 tokens.

Layout: activations are feature-major [feature partition-blocks, tokens].
Complex tensors are realified as separate real/imag feature planes.

Attention path runs in fp8e4m3 with DoubleRow matmuls (2 K-planes per pass):
x, qkv/v/proj weights and the attention output are fp8; softmax scores/exp
stay bf16/f32. LayerNorm1 is folded into the qkv weights via per-token stat
rows (mu_r, mu_i, std appended to the contraction) with the rstd applied at
PSUM eviction - this keeps the LN off the critical path. The MLP runs in bf16
(fp8 there fails the error budget): LayerNorm2 is materialized once (xh2) and
gelu reads PSUM directly with a fused per-feature bias. Attention scores are
computed transposed ([t2, t1]) so softmax sums reduce via ones-matmuls, and V
is produced pre-transposed by swapping matmul operands. All weights are stored
host-side in the exact SBUF layout so every weight DMA is fully contiguous.
"""
import sys
sys.path.insert(0, "/opt/trn_rl_repo")

from contextlib import ExitStack

import ml_dtypes
import numpy as np

import concourse.bacc as bacc
import concourse.bass as bass
import concourse.mybir as mybir
import concourse.tile as tile
from concourse.bass_utils import run_bass_kernel_spmd

# Prefer the table set that covers the whole softmax chain (square+ln+exp)
# so the greedy act-table-load pass doesn't thrash sets on every block.
_orig_get_tables = bacc.get_activation_tables


def _reordered_tables(arch):
    t = _orig_get_tables(arch)
    keep = {"natural_log_exp_and_others", "gelu_and_others"}
    return {k: (v if k in keep else set()) for k, v in t.items()}


bacc.get_activation_tables = _reordered_tables

dt = mybir.dt
AF = mybir.ActivationFunctionType
ALU = mybir.AluOpType
DR = mybir.MatmulPerfMode.DoubleRow
BF16 = ml_dtypes.bfloat16
F8 = ml_dtypes.float8_e4m3

B, N, C, H, DH, HID = 4, 1024, 768, 12, 64, 3072
NCORES = 8
OWN = 512          # tokens per core
SCALE = DH ** -0.5
EPS = 1e-5


def round_fp32r(x):
    b = np.ascontiguousarray(x, dtype=np.float32).view(np.uint32)
    lsb = (b >> np.uint32(12)) & np.uint32(1)
    return ((b + np.uint32(0x7FF) + lsb) & np.uint32(0xFFFFF000)).view(np.float32)


# --------------------------------------------------------------------------
# device program
# --------------------------------------------------------------------------

def build_nc(debug=False):
    nc = bacc.Bacc(trn_type="TRN2", target_bir_lowering=False)
    f32 = dt.float32
    f32r = dt.float32r
    bf16 = dt.bfloat16
    f8 = dt.float8e4

    # ---- DRAM I/O ----
    x_r = nc.dram_tensor("x_r", [128, 12, N], f8, kind="ExternalInput")
    x_own = nc.dram_tensor("x_own", [128, 12, OWN], f32, kind="ExternalInput")
    w_qkv = nc.dram_tensor("w_qkv", [H, 128, 12, 384], f8, kind="ExternalInput")
    w_qkv_s = nc.dram_tensor("w_qkv_s", [H, 4, 384], f32r, kind="ExternalInput")
    w_v = nc.dram_tensor("w_v", [6, 128, 12, 256], f8, kind="ExternalInput")
    w_v_s = nc.dram_tensor("w_v_s", [4, 1536], f32r, kind="ExternalInput")
    w_proj = nc.dram_tensor("w_proj", [12, 128, 12, 128], f8, kind="ExternalInput")
    w_pb = nc.dram_tensor("w_pb", [128, 12], f32, kind="ExternalInput")
    w_fc1r = nc.dram_tensor("w_fc1r", [24, 128, 6, 128], bf16, kind="ExternalInput")
    w_fc1i = nc.dram_tensor("w_fc1i", [24, 128, 6, 128], bf16, kind="ExternalInput")
    w_fc1in = nc.dram_tensor("w_fc1in", [24, 128, 6, 128], bf16, kind="ExternalInput")
    w_fc1b = nc.dram_tensor("w_fc1b", [128, 24, 2], f32, kind="ExternalInput")
    w_fc2r = nc.dram_tensor("w_fc2r", [6, 128, 24, 128], bf16, kind="ExternalInput")
    w_fc2i = nc.dram_tensor("w_fc2i", [6, 128, 24, 128], bf16, kind="ExternalInput")
    w_fc2in = nc.dram_tensor("w_fc2in", [6, 128, 24, 128], bf16, kind="ExternalInput")
    w_fc2_s = nc.dram_tensor("w_fc2_s", [6, 4, 256], f32r, kind="ExternalInput")
    ones_col = nc.dram_tensor("ones_col", [128, 1], bf16, kind="ExternalInput")
    ones_ab8 = nc.dram_tensor("ones_ab8", [128, 4], f8, kind="ExternalInput")
    ones_s8 = nc.dram_tensor("ones_s8", [128, 1], f8, kind="ExternalInput")
    ones_ab = nc.dram_tensor("ones_ab", [128, 4], bf16, kind="ExternalInput")
    ones_s = nc.dram_tensor("ones_s", [128, 1], bf16, kind="ExternalInput")
    stat_one = nc.dram_tensor("stat_one", [4, OWN], f32r, kind="ExternalInput")
    ident8 = nc.dram_tensor("ident8", [8, 8], f32r, kind="ExternalInput")

    out_fm = nc.dram_tensor("out_fm", [12, 128, OWN], f32, kind="ExternalOutput")

    with tile.TileContext(nc) as tc, ExitStack() as top:
        consts = top.enter_context(tc.tile_pool(name="consts", bufs=1))
        t_ones_col = consts.tile([128, 1], bf16)
        t_ones_ab8 = consts.tile([128, 4], f8)
        t_ones_s8 = consts.tile([128, 1], f8)
        t_ones_ab = consts.tile([128, 4], bf16)
        t_ones_s = consts.tile([128, 1], bf16)
        t_stat_one = consts.tile([4, OWN], f32r)
        t_id8 = consts.tile([8, 8], f32r)
        t_pb = consts.tile([128, 12], f32)
        t_f1b = consts.tile([128, 24, 2], f32)
        t_eps = consts.tile([1, 1], f32)
        nc.sync.dma_start(t_ones_col[:], ones_col[:])
        nc.sync.dma_start(t_ones_ab8[:], ones_ab8[:])
        nc.sync.dma_start(t_ones_s8[:], ones_s8[:])
        nc.sync.dma_start(t_ones_ab[:], ones_ab[:])
        nc.sync.dma_start(t_ones_s[:], ones_s[:])
        nc.sync.dma_start(t_stat_one[:], stat_one[:])
        nc.sync.dma_start(t_id8[:], ident8[:])
        nc.sync.dma_start(t_pb[:], w_pb[:])
        nc.sync.dma_start(t_f1b[:], w_fc1b[:])
        nc.vector.memset(t_eps[:], EPS)

        poolR1 = top.enter_context(tc.tile_pool(name="poolR1", bufs=1))
        xr1 = poolR1.tile([128, 12, OWN], f32, name="xr1")

        with ExitStack() as es_x:
            poolX = es_x.enter_context(tc.tile_pool(name="poolX", bufs=1))
            x8 = poolX.tile([128, 12, N], f8, name="x8")
            pdram = es_x.enter_context(
                tc.tile_pool(name="pdram", bufs=1, space="DRAM"))
            rstd_dram = pdram.tile([1, N], f32, name="rstd_dram")
            stat1s = [poolX.tile([4, 512], f32r, name=f"stat1_{ch}")
                      for ch in range(2)]
            rstd_bc1s = [poolX.tile([128, 512], f32, name=f"rstd_bc1_{ch}")
                         for ch in range(2)]
            rstdT = poolX.tile([128, 8], f32, name="rstdT")
            for kb in range(12):
                nc.sync.dma_start(x8[:, kb, :], x_r[:, kb, :])

            # ---------------- phase A: LN1 stats over full sequence --------
            with ExitStack() as es_a:
                pa = es_a.enter_context(tc.tile_pool(name="pa_sb", bufs=3))
                pa_ps = es_a.enter_context(
                    tc.tile_pool(name="pa_ps", bufs=2, space="PSUM"))
                pa_sc = es_a.enter_context(tc.tile_pool(name="pa_sc", bufs=2))
                mu_pss = [pa_ps.tile([2, 512], f32, tag=f"mu{ch}",
                                     name=f"mu{ch}", bufs=1) for ch in range(2)]
                s_pss = [pa_ps.tile([1, 512], f32, tag=f"s{ch}",
                                    name=f"s{ch}", bufs=1) for ch in range(2)]
                for kb in range(12):
                    sq = pa.tile([128, N], f8, tag="sq", name=f"sq{kb}")
                    nc.scalar.activation(sq[:], x8[:, kb, :], AF.Square)
                    lhs = t_ones_ab8[:, 0:2] if kb < 6 else t_ones_ab8[:, 2:4]
                    for ch in range(2):
                        sl = slice(ch * 512, ch * 512 + 512)
                        nc.tensor.matmul(mu_pss[ch][:], lhs, x8[:, kb, sl],
                                         start=(kb == 0), stop=(kb == 11))
                        nc.tensor.matmul(s_pss[ch][:], t_ones_s8[:], sq[:, sl],
                                         start=(kb == 0), stop=(kb == 11))
                for ch in range(2):
                    sl = slice(ch * 512, ch * 512 + 512)
                    mu_ps = mu_pss[ch]
                    s_ps = s_pss[ch]
                    # var = S - mu_r^2 - mu_i^2 ; std = exp(.5 ln(var+eps))
                    mu_sb = pa_sc.tile([2, 512], f32, tag="musb", name=f"musb{ch}")
                    mu_fl = pa_sc.tile([1, 2, 512], f32, tag="mufl", name=f"mufl{ch}")
                    var = pa_sc.tile([1, 512], f32, tag="var", name=f"var{ch}")
                    lnv = pa_sc.tile([1, 512], f32, tag="lnv", name=f"lnv{ch}")
                    s_c = pa_sc.tile([1, 512], f32, tag="sc_", name=f"sc_{ch}")
                    nc.vector.tensor_scalar(mu_sb[:], mu_ps[:], 1.0 / C, None,
                                            op0=ALU.mult)
                    nc.vector.tensor_scalar(s_c[:], s_ps[:], 1.0 / C, None,
                                            op0=ALU.mult)
                    nc.sync.dma_start(mu_fl[:, 0, :], mu_sb[0:1, :])
                    nc.sync.dma_start(mu_fl[:, 1, :], mu_sb[1:2, :])
                    sq_mu = pa_sc.tile([1, 2, 512], f32, tag="sqmu", name=f"sqmu{ch}")
                    nc.vector.tensor_tensor(sq_mu[:], mu_fl[:], mu_fl[:],
                                            op=ALU.mult)
                    nc.vector.tensor_tensor(var[:], s_c[:], sq_mu[:, 0, :],
                                            op=ALU.subtract)
                    nc.vector.tensor_tensor(var[:], var[:], sq_mu[:, 1, :],
                                            op=ALU.subtract)
                    nc.scalar.activation(lnv[:], var[:], AF.Ln, bias=t_eps[:])
                    # stats rows: 0=mu_r 1=mu_i 2=std
                    nc.vector.tensor_copy(stat1s[ch][0:2, :], mu_sb[:])
                    std_row = pa_sc.tile([1, 512], f32r, tag="stdr", name=f"stdr{ch}")
                    nc.scalar.activation(std_row[:], lnv[:], AF.Exp, scale=0.5)
                    nc.sync.dma_start(stat1s[ch][2:3, :], std_row[:])
                    rstd_row = pa_sc.tile([1, 512], f32r, tag="rst", name=f"rst{ch}")
                    nc.scalar.activation(rstd_row[:], lnv[:], AF.Exp, scale=-0.5)
                    nc.sync.dma_start(rstd_dram[:, sl], rstd_row[:].bitcast(f32))
                    bcast = bass.AP(tensor=rstd_dram.tensor,
                                    offset=rstd_dram[:, sl].offset,
                                    ap=[[0, 128]] + rstd_dram[:, sl].ap[1:])
                    nc.sync.dma_start(rstd_bc1s[ch][:], bcast)
                # rstd transposed: rstdT[p, t2b] = rstd[t2b*128 + p]
                rstd8 = pa_sc.tile([8, 128], f32, tag="r8", name="rstd8")
                nc.sync.dma_start(
                    rstd8[:], rstd_dram[:].rearrange("o (a b) -> (o a) b", a=8))
                rstdT_ps = pa_ps.tile([128, 8], f32, tag="rtps", name="rtps")
                nc.tensor.transpose(rstdT_ps[:], rstd8[:], t_id8[:].bitcast(f32))
                nc.vector.tensor_copy(rstdT[:], rstdT_ps[:])

            # ---------------- phase BC: qkv + attention per head ----------
            es_attn = ExitStack()
            attnp = es_attn.enter_context(tc.tile_pool(name="attnp", bufs=1))
            attn = attnp.tile([128, 12, OWN], f8, name="attn")
            es_b = ExitStack()
            pq = es_b.enter_context(tc.tile_pool(name="pq", bufs=1))
            pk = es_b.enter_context(tc.tile_pool(name="pk", bufs=1))
            pvt = es_b.enter_context(tc.tile_pool(name="pvt", bufs=2))
            pwv = es_b.enter_context(tc.tile_pool(name="pwv", bufs=1))
            pwq = es_b.enter_context(tc.tile_pool(name="pwq", bufs=2))
            pet = es_b.enter_context(tc.tile_pool(name="pet", bufs=3))
            psc = es_b.enter_context(tc.tile_pool(name="psc", bufs=4))
            prd = es_b.enter_context(tc.tile_pool(name="prd", bufs=2))
            ps_rot = es_b.enter_context(
                tc.tile_pool(name="ps_rot", bufs=2, space="PSUM"))
            ps_sc = es_b.enter_context(
                tc.tile_pool(name="ps_sc", bufs=2, space="PSUM"))
            ps_acc = es_b.enter_context(
                tc.tile_pool(name="ps_acc", bufs=2, space="PSUM"))
            pdram_rd = es_b.enter_context(
                tc.tile_pool(name="pdram_rd", bufs=2, space="DRAM"))
            vt_pair = None
            et_fifo = []
            acc_ps = {}
            LAG = 4

            def emit_avden(ent):
                h2, t2b2, et2, vt2 = ent
                slot2 = h2 % 2
                if t2b2 == 0:
                    acc_ps[h2] = (
                        ps_acc.tile([128, OWN], f32, tag="av", name=f"av{h2}",
                                    bufs=1),
                        ps_acc.tile([1, OWN], f32, tag="den", name=f"den{h2}",
                                    bufs=1),
                    )
                av2, den2 = acc_ps[h2]
                nc.tensor.matmul(den2[:], t_ones_col[:], et2,
                                 start=(t2b2 == 0), stop=(t2b2 == 7))
                dsl2 = slice(slot2 * 128, slot2 * 128 + 128)
                nc.tensor.matmul(av2[:], vt2[:, t2b2, dsl2], et2,
                                 start=(t2b2 == 0), stop=(t2b2 == 7))
                if t2b2 == 7:
                    den_sb = prd.tile([1, OWN], f32, tag="den_sb",
                                      name=f"dsb{h2}", bufs=1)
                    nc.vector.tensor_copy(den_sb[:], den2[:])
                    den_dram = pdram_rd.tile([1, OWN], f32, tag="dend",
                                             name=f"dend{h2}")
                    nc.sync.dma_start(den_dram[:], den_sb[:])
                    den_sp = prd.tile([128, 4], f32, tag="den_sp",
                                      name=f"dsp{h2}", bufs=1)
                    nc.sync.dma_start(
                        den_sp[:],
                        den_dram[:].rearrange("o (a b) -> (o a) b", a=128))
                    rd_sp = prd.tile([128, 4], f32, tag="rd_sp",
                                     name=f"rsp{h2}", bufs=1)
                    nc.vector.reciprocal(rd_sp[:], den_sp[:])
                    rd_dram = pdram_rd.tile([1, OWN], f32, tag="rdd",
                                            name=f"rdd{h2}")
                    nc.sync.dma_start(
                        rd_dram[:].rearrange("o (a b) -> (o a) b", a=128),
                        rd_sp[:])
                    rd_bc = prd.tile([128, OWN], f32, tag="rd_bc",
                                     name=f"rdbc{h2}", bufs=1)
                    rd_bcast_ap = bass.AP(tensor=rd_dram.tensor,
                                          offset=rd_dram[:].offset,
                                          ap=[[0, 128]] + rd_dram[:].ap[1:])
                    nc.sync.dma_start(rd_bc[:], rd_bcast_ap)
                    nc.vector.tensor_tensor(attn[:, h2, :], av2[:], rd_bc[:],
                                            op=ALU.mult)
                    del acc_ps[h2]

            for h in range(H):
                pair, slot = divmod(h, 2)
                # qkv for head h: q1=[q_r;-q_i], q3=[q_i;q_r], k=[k_r;k_i]
                q_t = pq.tile([128, 2, OWN], bf16, tag="q", name=f"q{h}")
                k_t = pk.tile([128, N], bf16, tag="k", name=f"k{h}")
                wqkv_t = pwq.tile([128, 12, 384], f8, tag="wqkv",
                                  name=f"wqkv{h}")
                wqs_t = pwq.tile([4, 384], f32r, tag="wqs", name=f"wqs{h}")
                nc.sync.dma_start(wqkv_t[:], w_qkv[h])
                nc.sync.dma_start(wqs_t[:], w_qkv_s[h])
                q1_ps = ps_rot.tile([128, OWN], f32, tag="rot", name=f"q1ps{h}")
                q3_ps = ps_rot.tile([128, OWN], f32, tag="rot", name=f"q3ps{h}")
                for p in range(6):
                    kp = slice(2 * p, 2 * p + 2)
                    st = (p == 0)
                    nc.tensor.matmul(q1_ps[:], wqkv_t[:, kp, 0:128],
                                     x8[:, kp, 0:OWN], start=st, stop=False,
                                     perf_mode=DR)
                    nc.tensor.matmul(q3_ps[:], wqkv_t[:, kp, 128:256],
                                     x8[:, kp, 0:OWN], start=st, stop=False,
                                     perf_mode=DR)
                nc.tensor.matmul(q1_ps[:], wqs_t[:, 0:128], stat1s[0][:],
                                 start=False, stop=True)
                nc.tensor.matmul(q3_ps[:], wqs_t[:, 128:256], stat1s[0][:],
                                 start=False, stop=True)
                nc.vector.tensor_tensor(q_t[:, 0, :], q1_ps[:],
                                        rstd_bc1s[0][:], op=ALU.mult)
                nc.vector.tensor_tensor(q_t[:, 1, :], q3_ps[:],
                                        rstd_bc1s[0][:], op=ALU.mult)
                k0_ps = ps_rot.tile([128, 512], f32, tag="rot", name=f"k0ps{h}")
                k1_ps = ps_rot.tile([128, 512], f32, tag="rot", name=f"k1ps{h}")
                for p in range(6):
                    kp = slice(2 * p, 2 * p + 2)
                    st = (p == 0)
                    nc.tensor.matmul(k0_ps[:], wqkv_t[:, kp, 256:384],
                                     x8[:, kp, 0:512], start=st, stop=False,
                                     perf_mode=DR)
                    nc.tensor.matmul(k1_ps[:], wqkv_t[:, kp, 256:384],
                                     x8[:, kp, 512:N], start=st, stop=False,
                                     perf_mode=DR)
                nc.tensor.matmul(k0_ps[:], wqs_t[:, 256:384], stat1s[0][:],
                                 start=False, stop=True)
                nc.tensor.matmul(k1_ps[:], wqs_t[:, 256:384], stat1s[1][:],
                                 start=False, stop=True)
                nc.vector.tensor_tensor(k_t[:, 0:512], k0_ps[:],
                                        rstd_bc1s[0][:], op=ALU.mult)
                nc.vector.tensor_tensor(k_t[:, 512:N], k1_ps[:],
                                        rstd_bc1s[1][:], op=ALU.mult)
                if slot == 0:
                    # V^T for this head pair: [t2, d] via swapped operands
                    wv_t = pwv.tile([128, 12, 256], f8, tag="wv",
                                    name=f"wv{pair}")
                    wv_s = pwv.tile([4, 256], f32r, tag="wvs",
                                    name=f"wvs{pair}")
                    csl = slice(pair * 256, pair * 256 + 256)
                    nc.sync.dma_start(wv_t[:], w_v[pair])
                    nc.sync.dma_start(wv_s[:], w_v_s[:, csl])
                    vt_pair = pvt.tile([128, 8, 256], bf16, tag="vt",
                                       name=f"vt{pair}")
                    for t2b in range(8):
                        t2s = slice(t2b * 128, t2b * 128 + 128)
                        vt_ps = ps_rot.tile([128, 256], f32, tag="rot",
                                            name=f"vtps{pair}_{t2b}")
                        for p in range(6):
                            kp = slice(2 * p, 2 * p + 2)
                            nc.tensor.matmul(vt_ps[:], x8[:, kp, t2s],
                                             wv_t[:, kp, :],
                                             start=(p == 0), stop=False,
                                             perf_mode=DR)
                        st1 = stat1s[t2b // 4]
                        t2l = slice((t2b % 4) * 128, (t2b % 4) * 128 + 128)
                        nc.tensor.matmul(vt_ps[:], st1[:, t2l], wv_s[:],
                                         start=False, stop=True)
                        nc.vector.tensor_scalar(
                            vt_pair[:, t2b, :], vt_ps[:],
                            rstdT[:, t2b:t2b + 1], None, op0=ALU.mult)
                # scores + exp chain, batched over block pairs;
                # den/av matmuls lag by LAG sub-blocks
                for t2p in range(4):
                    t2s0 = slice(t2p * 256, t2p * 256 + 128)
                    t2s1 = slice(t2p * 256 + 128, t2p * 256 + 256)
                    sr_pair = ps_sc.tile([128, 2, OWN], f32, tag="scp",
                                         name=f"srp{h}_{t2p}")
                    si_pair = ps_sc.tile([128, 2, OWN], f32, tag="scp",
                                         name=f"sip{h}_{t2p}")
                    nc.tensor.matmul(sr_pair[:, 0, :], k_t[:, t2s0],
                                     q_t[:, 0, :], start=True, stop=True)
                    nc.tensor.matmul(si_pair[:, 0, :], k_t[:, t2s0],
                                     q_t[:, 1, :], start=True, stop=True)
                    nc.tensor.matmul(sr_pair[:, 1, :], k_t[:, t2s1],
                                     q_t[:, 0, :], start=True, stop=True)
                    nc.tensor.matmul(si_pair[:, 1, :], k_t[:, t2s1],
                                     q_t[:, 1, :], start=True, stop=True)
                    sqr = psc.tile([128, 2, OWN], f32, tag="sqr",
                                   name=f"sqr{h}_{t2p}")
                    sqi = psc.tile([128, 2, OWN], f32, tag="sqi",
                                   name=f"sqi{h}_{t2p}")
                    if t2p % 2 == 0:
                        m2q = psc.tile([128, 4, OWN], f32, tag="m2q",
                                       name=f"m2q{h}_{t2p // 2}")
                        etq = pet.tile([128, 4, OWN], bf16, tag="et",
                                       name=f"et{h}_{t2p // 2}")
                    nc.scalar.activation(sqr[:], sr_pair[:], AF.Square)
                    nc.scalar.activation(sqi[:], si_pair[:], AF.Square)
                    qsl = slice(2 * (t2p % 2), 2 * (t2p % 2) + 2)
                    nc.vector.tensor_tensor(m2q[:, qsl, :], sqr[:], sqi[:],
                                            op=ALU.add)
                    if t2p % 2 == 1:
                        # chain over the 4-block quad: ln -> 0.5ln -> mag -> exp
                        nc.scalar.activation(m2q[:], m2q[:], AF.Ln)
                        nc.scalar.activation(m2q[:], m2q[:], AF.Exp, scale=0.5)
                        nc.scalar.activation(etq[:], m2q[:], AF.Exp)
                        for sub in range(4):
                            et_fifo.append((h, (t2p - 1) * 2 + sub,
                                            etq[:, sub, :], vt_pair))
                            while len(et_fifo) > LAG:
                                emit_avden(et_fifo.pop(0))
            for ent in et_fifo:
                emit_avden(ent)
            et_fifo.clear()
            es_b.close()

            # ------------- phase D: proj + residual --------------------
            nc.sync.dma_start(xr1[:], x_own[:])
            for opb in range(12):
                nc.vector.tensor_scalar(xr1[:, opb, :], xr1[:, opb, :],
                                        t_pb[:, opb:opb + 1], None,
                                        op0=ALU.add)
            r1r = poolR1.tile([128, 12, OWN], bf16, name="r1r")
            with ExitStack() as es_d:
                pwp = es_d.enter_context(tc.tile_pool(name="pwp", bufs=3))
                ps_d = es_d.enter_context(
                    tc.tile_pool(name="ps_d", bufs=4, space="PSUM"))
                for opb in range(12):
                    wp_t = pwp.tile([128, 12, 128], f8, tag="wp",
                                    name=f"wp{opb}")
                    nc.sync.dma_start(wp_t[:], w_proj[opb])
                    pr_ps = ps_d.tile([128, OWN], f32, tag="pr",
                                      name=f"prps{opb}")
                    for p in range(6):
                        kp = slice(2 * p, 2 * p + 2)
                        nc.tensor.matmul(pr_ps[:], wp_t[:, kp, :],
                                         attn[:, kp, :],
                                         start=(p == 0), stop=(p == 5),
                                         perf_mode=DR)
                    nc.vector.tensor_tensor(xr1[:, opb, :], pr_ps[:],
                                            xr1[:, opb, :], op=ALU.add)
                    nc.vector.tensor_copy(r1r[:, opb, :], xr1[:, opb, :])
            es_attn.close()

        # ---------------- phase E: LN2 stats + normalized r1 --------------
        poolE = top.enter_context(tc.tile_pool(name="poolE", bufs=1))
        xh2 = poolE.tile([128, 12, OWN], bf16, name="xh2")
        with ExitStack() as es_e:
            pe = es_e.enter_context(tc.tile_pool(name="pe_sb", bufs=1))
            pdram2 = es_e.enter_context(
                tc.tile_pool(name="pdram2", bufs=1, space="DRAM"))
            pe_ps = es_e.enter_context(
                tc.tile_pool(name="pe_ps", bufs=2, space="PSUM"))
            sq2s = []
            for kb in range(12):
                sq2 = pe.tile([128, OWN], bf16, tag="sq2", name=f"sq2_{kb}", bufs=12)
                nc.scalar.activation(sq2[:], r1r[:, kb, :], AF.Square)
                sq2s.append(sq2)
            mu2_ps = pe_ps.tile([2, OWN], f32, tag="mu2", name="mu2")
            s2_ps = pe_ps.tile([1, OWN], f32, tag="s2", name="s2")
            for kb in range(12):
                lhs = t_ones_ab[:, 0:2] if kb < 6 else t_ones_ab[:, 2:4]
                nc.tensor.matmul(mu2_ps[:], lhs, r1r[:, kb, :],
                                 start=(kb == 0), stop=(kb == 11))
                nc.tensor.matmul(s2_ps[:], t_ones_s[:], sq2s[kb][:],
                                 start=(kb == 0), stop=(kb == 11))
            mu2_sb = pe.tile([2, OWN], f32, tag="emusb", name="emusb")
            mu2_fl = pe.tile([1, 2, OWN], f32, tag="emufl", name="emufl")
            var = pe.tile([1, OWN], f32, tag="evar", name="evar")
            lnv = pe.tile([1, OWN], f32, tag="elnv", name="elnv")
            s2_c = pe.tile([1, OWN], f32, tag="es2c", name="es2c")
            nc.vector.tensor_scalar(mu2_sb[:], mu2_ps[:], 1.0 / C, None,
                                    op0=ALU.mult)
            nc.vector.tensor_scalar(s2_c[:], s2_ps[:], 1.0 / C, None,
                                    op0=ALU.mult)
            nc.sync.dma_start(mu2_fl[:, 0, :], mu2_sb[0:1, :])
            nc.sync.dma_start(mu2_fl[:, 1, :], mu2_sb[1:2, :])
            sq_mu2 = pe.tile([1, 2, OWN], f32, tag="esqmu", name="esqmu")
            nc.vector.tensor_tensor(sq_mu2[:], mu2_fl[:], mu2_fl[:], op=ALU.mult)
            nc.vector.tensor_tensor(var[:], s2_c[:], sq_mu2[:, 0, :],
                                    op=ALU.subtract)
            nc.vector.tensor_tensor(var[:], var[:], sq_mu2[:, 1, :],
                                    op=ALU.subtract)
            nc.scalar.activation(lnv[:], var[:], AF.Ln, bias=t_eps[:])
            rstd2_row = pe.tile([1, OWN], f32, tag="ers", name="ers")
            nc.scalar.activation(rstd2_row[:], lnv[:], AF.Exp, scale=-0.5)
            stat2_dram = pdram2.tile([4, OWN], f32, name="stat2_dram")
            nc.sync.dma_start(stat2_dram[0:1, :], mu2_sb[0:1, :])
            nc.sync.dma_start(stat2_dram[1:2, :], mu2_sb[1:2, :])
            nc.sync.dma_start(stat2_dram[2:3, :], rstd2_row[:])
            mu2r_bc = pe.tile([128, OWN], f32, tag="m2rbc", name="m2rbc")
            mu2i_bc = pe.tile([128, OWN], f32, tag="m2ibc", name="m2ibc")
            rstd2_bc = pe.tile([128, OWN], f32, tag="r2bc", name="r2bc")
            for row, t in ((0, mu2r_bc), (1, mu2i_bc), (2, rstd2_bc)):
                rr = stat2_dram[row:row + 1, :]
                nc.sync.dma_start(t[:], bass.AP(
                    tensor=stat2_dram.tensor, offset=rr.offset,
                    ap=[[0, 128]] + rr.ap[1:]))
            for kb in range(12):
                mbc = mu2r_bc if kb < 6 else mu2i_bc
                nc.vector.tensor_tensor(xh2[:, kb, :], r1r[:, kb, :],
                                        mbc[:], op=ALU.subtract)
                nc.vector.tensor_tensor(xh2[:, kb, :], xh2[:, kb, :],
                                        rstd2_bc[:], op=ALU.mult)

        # ---------------- phase F: MLP, single 512-token pass -------------
        with ExitStack() as es_f:
            ph = es_f.enter_context(tc.tile_pool(name="ph", bufs=1))
            pw1 = es_f.enter_context(tc.tile_pool(name="pw1", bufs=3))
            pw2 = es_f.enter_context(tc.tile_pool(name="pw2", bufs=3))
            pout = es_f.enter_context(tc.tile_pool(name="pout", bufs=2))
            ps_f = es_f.enter_context(
                tc.tile_pool(name="ps_f", bufs=4, space="PSUM"))
            h_t = ph.tile([128, 48, OWN], bf16, name="h_t")
            for Cb in range(24):
                w1r_t = pw1.tile([128, 6, 128], bf16, tag="w1r",
                                 name=f"w1r{Cb}")
                w1i_t = pw1.tile([128, 6, 128], bf16, tag="w1i",
                                 name=f"w1i{Cb}")
                w1in_t = pw1.tile([128, 6, 128], bf16, tag="w1in",
                                  name=f"w1in{Cb}")
                nc.sync.dma_start(w1r_t[:], w_fc1r[Cb])
                nc.sync.dma_start(w1i_t[:], w_fc1i[Cb])
                nc.sync.dma_start(w1in_t[:], w_fc1in[Cb])
                hr_ps = ps_f.tile([128, OWN], f32, tag="fps",
                                  name=f"hrps{Cb}")
                hi_ps = ps_f.tile([128, OWN], f32, tag="fps",
                                  name=f"hips{Cb}")
                for kb in range(6):
                    st = (kb == 0)
                    nc.tensor.matmul(hr_ps[:], w1r_t[:, kb, :],
                                     xh2[:, kb, :], start=st, stop=False)
                    nc.tensor.matmul(hi_ps[:], w1i_t[:, kb, :],
                                     xh2[:, kb, :], start=st, stop=False)
                for kb in range(6):
                    lst = (kb == 5)
                    nc.tensor.matmul(hr_ps[:], w1in_t[:, kb, :],
                                     xh2[:, 6 + kb, :], start=False,
                                     stop=lst)
                    nc.tensor.matmul(hi_ps[:], w1r_t[:, kb, :],
                                     xh2[:, 6 + kb, :], start=False,
                                     stop=lst)
                nc.scalar.activation(h_t[:, Cb, :], hr_ps[:], AF.Gelu,
                                     bias=t_f1b[:, Cb, 0:1])
                nc.scalar.activation(h_t[:, 24 + Cb, :], hi_ps[:], AF.Gelu,
                                     bias=t_f1b[:, Cb, 1:2])
            for j in range(6):
                w2r_t = pw2.tile([128, 24, 128], bf16, tag="w2r",
                                 name=f"w2r{j}")
                w2i_t = pw2.tile([128, 24, 128], bf16, tag="w2i",
                                 name=f"w2i{j}")
                w2in_t = pw2.tile([128, 24, 128], bf16, tag="w2in",
                                  name=f"w2in{j}")
                w2s_t = pw2.tile([4, 256], f32r, tag="w2s",
                                 name=f"w2s{j}")
                nc.sync.dma_start(w2r_t[:], w_fc2r[j])
                nc.sync.dma_start(w2i_t[:], w_fc2i[j])
                nc.sync.dma_start(w2in_t[:], w_fc2in[j])
                nc.sync.dma_start(w2s_t[:], w_fc2_s[j])
                or_ps = ps_f.tile([128, OWN], f32, tag="fps",
                                  name=f"orps{j}")
                oi_ps = ps_f.tile([128, OWN], f32, tag="fps",
                                  name=f"oips{j}")
                for kb in range(24):
                    st = (kb == 0)
                    nc.tensor.matmul(or_ps[:], w2r_t[:, kb, :], h_t[:, kb, :],
                                     start=st, stop=False)
                    nc.tensor.matmul(oi_ps[:], w2i_t[:, kb, :], h_t[:, kb, :],
                                     start=st, stop=False)
                for kb in range(24):
                    nc.tensor.matmul(or_ps[:], w2in_t[:, kb, :],
                                     h_t[:, 24 + kb, :],
                                     start=False, stop=False)
                    nc.tensor.matmul(oi_ps[:], w2r_t[:, kb, :],
                                     h_t[:, 24 + kb, :],
                                     start=False, stop=False)
                nc.tensor.matmul(or_ps[:], w2s_t[:, 0:128],
                                 t_stat_one[:], start=False, stop=True)
                nc.tensor.matmul(oi_ps[:], w2s_t[:, 128:256],
                                 t_stat_one[:], start=False, stop=True)
                o_r = pout.tile([128, OWN], f32, tag="o", name=f"or{j}")
                o_i = pout.tile([128, OWN], f32, tag="o", name=f"oi{j}")
                nc.vector.tensor_tensor(o_r[:], or_ps[:], xr1[:, j, :],
                                        op=ALU.add)
                nc.vector.tensor_tensor(o_i[:], oi_ps[:], xr1[:, 6 + j, :],
                                        op=ALU.add)
                nc.sync.dma_start(out_fm[j], o_r[:])
                nc.sync.dma_start(out_fm[6 + j], o_i[:])
    nc.compile()
    return nc


# --------------------------------------------------------------------------
# host side
# --------------------------------------------------------------------------

def _cx(a):
    return a[..., 0].astype(np.float64) + 1j * a[..., 1].astype(np.float64)


def _kcols(Wp, wsum, wb, plane, scale=1.0):
    """K-profile [1539, m] for output features with complex weight rows Wp
    [m, 768], LN fold sums wsum [m], bias-column wb [m]. K rows: xr(768),
    xi(768), mu_r, mu_i, std."""
    m = Wp.shape[0]
    out = np.zeros((1539, m), np.float64)
    if plane == "r":
        out[0:768] = Wp.real.T
        out[768:1536] = -Wp.imag.T
        out[1536] = -wsum.real
        out[1537] = wsum.imag
        out[1538] = wb.real
    else:
        out[0:768] = Wp.imag.T
        out[768:1536] = Wp.real.T
        out[1536] = -wsum.imag
        out[1537] = -wsum.real
        out[1538] = wb.imag
    return out * scale


def _bf(a):
    return np.ascontiguousarray(a).astype(BF16)


def _f8(a):
    return np.ascontiguousarray(a).astype(F8)


def _pmajor(a):
    """[kb, 128, n] -> [128, kb, n] partition-major contiguous."""
    return np.ascontiguousarray(np.transpose(a, (1, 0, 2)))


def _prep_weights(inputs):
    n1 = _cx(inputs["n1_w"]); b1 = _cx(inputs["n1_b"])
    n2 = _cx(inputs["n2_w"]); b2 = _cx(inputs["n2_b"])
    Wqkv = _cx(inputs["qkv_w"])          # [2304, 768]
    Wp = _cx(inputs["proj_w"])           # [768, 768]
    bp = _cx(inputs["proj_b"])           # [768]
    W1 = _cx(inputs["fc1_w"])            # [3072, 768]
    bf1 = _cx(inputs["fc1_b"])           # [3072]
    W2 = _cx(inputs["fc2_w"])            # [768, 3072]
    bf2 = _cx(inputs["fc2_b"])           # [768]

    d = {}
    # ---- qkv (LN1-folded) ----
    Wq, Wk, Wv = Wqkv[0:768], Wqkv[768:1536], Wqkv[1536:2304]

    def fold1(W):
        Wf = W * n1[None, :]
        return Wf, Wf.sum(1), W @ b1

    w_qkv = np.zeros((H, 128, 12, 384), F8)
    w_qkv_s = np.zeros((H, 4, 384), np.float32)
    for h in range(H):
        rows = slice(h * DH, (h + 1) * DH)
        Qf, Qs, Qb = fold1(Wq[rows])
        Kf, Ks, Kb_ = fold1(Wk[rows])
        q1 = np.hstack([_kcols(Qf, Qs, Qb, "r", SCALE),
                        _kcols(Qf, Qs, Qb, "i", -SCALE)])
        q3 = np.hstack([_kcols(Qf, Qs, Qb, "i", SCALE),
                        _kcols(Qf, Qs, Qb, "r", SCALE)])
        kk = np.hstack([_kcols(Kf, Ks, Kb_, "r"), _kcols(Kf, Ks, Kb_, "i")])
        blk = np.hstack([q1, q3, kk]).astype(np.float32)       # [1539, 384]
        w_qkv[h] = _f8(_pmajor(blk[0:1536].reshape(12, 128, 384)))
        w_qkv_s[h, 0:3] = blk[1536:1539]
    d["w_qkv"] = w_qkv
    d["w_qkv_s"] = round_fp32r(w_qkv_s)

    # ---- v (LN1-folded), rhs layout; cols: pair*256+slot*128+plane*64+dh
    wv_full = np.zeros((1539, 1536), np.float64)
    for h in range(H):
        rows = slice(h * DH, (h + 1) * DH)
        Vf, Vs, Vb = fold1(Wv[rows])
        base = h * 128
        wv_full[:, base:base + 64] = _kcols(Vf, Vs, Vb, "r")
        wv_full[:, base + 64:base + 128] = _kcols(Vf, Vs, Vb, "i")
    w_v = np.zeros((6, 128, 12, 256), F8)
    for pair in range(6):
        csl = slice(pair * 256, pair * 256 + 256)
        w_v[pair] = _f8(_pmajor(wv_full[0:1536, csl].reshape(12, 128, 256)))
    d["w_v"] = w_v
    wvs = np.zeros((4, 1536), np.float32)
    wvs[0:3] = wv_full[1536:1539]
    d["w_v_s"] = round_fp32r(wvs)

    # ---- proj; K rows = attn features: per head [a_r(64); a_i(64)] ----
    w_proj = np.zeros((12, 128, 12, 128), F8)
    w_pb = np.zeros((128, 12), np.float32)
    for opb in range(12):
        plane = "r" if opb < 6 else "i"
        orow = slice((opb % 6) * 128, (opb % 6) * 128 + 128)
        Wpo = Wp[orow]                               # [128, 768] complex
        prof = np.zeros((1536, 128), np.float64)
        for hh in range(H):
            cols = slice(hh * DH, (hh + 1) * DH)
            if plane == "r":
                prof[hh * 128:hh * 128 + 64] = Wpo.real[:, cols].T
                prof[hh * 128 + 64:hh * 128 + 128] = -Wpo.imag[:, cols].T
            else:
                prof[hh * 128:hh * 128 + 64] = Wpo.imag[:, cols].T
                prof[hh * 128 + 64:hh * 128 + 128] = Wpo.real[:, cols].T
        w_proj[opb] = _f8(_pmajor(prof.reshape(12, 128, 128)))
        w_pb[:, opb] = (bp.real if plane == "r" else bp.imag)[orow]
    d["w_proj"] = w_proj
    d["w_pb"] = w_pb

    # ---- fc1 (gain-folded; bias separate; LN2 applied via xh2) ----
    W1f = W1 * n2[None, :]
    W1b = W1 @ b2 + bf1
    w_fc1r = np.zeros((24, 128, 6, 128), BF16)
    w_fc1i = np.zeros((24, 128, 6, 128), BF16)
    w_fc1in = np.zeros((24, 128, 6, 128), BF16)
    w_fc1b = np.zeros((128, 24, 2), np.float32)
    for Cb in range(24):
        orow = slice(Cb * 128, (Cb + 1) * 128)
        tr = np.zeros((6, 128, 128), np.float64)
        ti = np.zeros((6, 128, 128), np.float64)
        for kb in range(6):
            icol = slice(kb * 128, (kb + 1) * 128)
            tr[kb] = W1f.real[orow, icol].T
            ti[kb] = W1f.imag[orow, icol].T
        w_fc1r[Cb] = _bf(_pmajor(tr))
        w_fc1i[Cb] = _bf(_pmajor(ti))
        w_fc1in[Cb] = _bf(_pmajor(-ti))
        w_fc1b[:, Cb, 0] = W1b.real[orow]
        w_fc1b[:, Cb, 1] = W1b.imag[orow]
    d["w_fc1r"] = w_fc1r
    d["w_fc1i"] = w_fc1i
    d["w_fc1in"] = w_fc1in
    d["w_fc1b"] = w_fc1b

    # ---- fc2 (plain + bias) ----
    w_fc2r = np.zeros((6, 128, 24, 128), BF16)
    w_fc2i = np.zeros((6, 128, 24, 128), BF16)
    w_fc2in = np.zeros((6, 128, 24, 128), BF16)
    w_fc2_s = np.zeros((6, 4, 256), np.float32)
    for j in range(6):
        orow = slice(j * 128, (j + 1) * 128)
        tr = np.zeros((24, 128, 128), np.float64)
        ti = np.zeros((24, 128, 128), np.float64)
        for kb in range(24):
            icol = slice(kb * 128, (kb + 1) * 128)
            tr[kb] = W2.real[orow, icol].T
            ti[kb] = W2.imag[orow, icol].T
        w_fc2r[j] = _bf(_pmajor(tr))
        w_fc2i[j] = _bf(_pmajor(ti))
        w_fc2in[j] = _bf(_pmajor(-ti))
        w_fc2_s[j, 0, 0:128] = bf2.real[orow]
        w_fc2_s[j, 0, 128:256] = bf2.imag[orow]
    d["w_fc2r"] = w_fc2r
    d["w_fc2i"] = w_fc2i
    d["w_fc2in"] = w_fc2in
    d["w_fc2_s"] = round_fp32r(w_fc2_s)

    # ---- consts ----
    d["ones_col"] = np.ones((128, 1), BF16)
    oab = np.zeros((128, 4), np.float32)
    oab[:, 0] = 1.0
    oab[:, 3] = 1.0
    d["ones_ab8"] = oab.astype(F8)
    d["ones_s8"] = np.ones((128, 1), F8)
    d["ones_ab"] = oab.astype(BF16)
    d["ones_s"] = np.ones((128, 1), BF16)
    so = np.zeros((4, OWN), np.float32)
    so[0] = 1.0
    d["stat_one"] = so
    d["ident8"] = np.eye(8, dtype=np.float32)
    return d


_NC_CACHE = {}


def kernel(**inputs):
    debug = bool(inputs.pop("_debug", False))
    if debug not in _NC_CACHE:
        _NC_CACHE[debug] = build_nc(debug=debug)
    nc = _NC_CACHE[debug]

    shared = _prep_weights(inputs)
    x = np.asarray(inputs["x"], np.float32)          # [B, N, C, 2]

    in_maps = []
    for c in range(NCORES):
        b, half = divmod(c, 2)
        xr_ = x[b, :, :, 0].T                        # [768, 1024]
        xi_ = x[b, :, :, 1].T
        stack = np.concatenate([xr_, xi_], 0)        # [1536, 1024]
        if half == 1:
            stack = np.concatenate([stack[:, OWN:], stack[:, :OWN]], 1)
        m = dict(shared)
        m["x_r"] = np.ascontiguousarray(
            stack.reshape(12, 128, N).transpose(1, 0, 2)).astype(F8)
        m["x_own"] = np.ascontiguousarray(
            stack[:, 0:OWN].reshape(12, 128, OWN).transpose(1, 0, 2))
        in_maps.append(m)

    res = run_bass_kernel_spmd(nc, in_maps, list(range(NCORES)))
    out = np.empty((B, N, C, 2), np.float32)
    for c in range(NCORES):
        b, half = divmod(c, 2)
        o = res.results[c]["out_fm"]                 # [12, 128, OWN]
        sl = slice(half * OWN, half * OWN + OWN)
        out[b, sl, :, 0] = o[0:6].reshape(768, OWN).T
        out[b, sl, :, 1] = o[6:12].reshape(768, OWN).T
    if debug:
        return out, res
    return out


# revision 17
# speedup vs baseline: 1.3016x; 1.0083x over previous
"""Complex transformer block (LN->attn->LN->MLP, complex arithmetic) on 8 TRN2 cores.

Sharding: core c handles (batch b = c//2, sequence half = c%2). No collectives:
each core computes K/V over the full 1024-token sequence of its batch (the only
duplicated work) and queries/MLP over its own 512 tokens.

Layout: activations are feature-major [feature partition-blocks, tokens].
Complex tensors are realified as separate real/imag feature planes.

Attention path runs in fp8e4m3 with DoubleRow matmuls (2 K-planes per pass):
x, qkv/v/proj weights and the attention output are fp8; softmax scores/exp
stay bf16/f32. LayerNorm1 is folded into the qkv weights via per-token stat
rows (mu_r, mu_i, std appended to the contraction) with the rstd applied at
PSUM eviction - this keeps the LN off the critical path. The MLP runs in bf16
(fp8 there fails the error budget): LayerNorm2 is materialized once (xh2) and
gelu reads PSUM directly with a fused per-feature bias. Attention scores are
computed transposed ([t2, t1]) so softmax sums reduce via ones-matmuls, and V
is produced pre-transposed by swapping matmul operands. All weights are stored
host-side in the exact SBUF layout so every weight DMA is fully contiguous.
"""
import sys
sys.path.insert(0, "/opt/trn_rl_repo")

from contextlib import ExitStack

import ml_dtypes
import numpy as np

import concourse.bacc as bacc
import concourse.bass as bass
import concourse.mybir as mybir
import concourse.tile as tile
from concourse.bass_utils import run_bass_kernel_spmd

# Prefer the table set that covers the whole softmax chain (square+ln+exp)
# so the greedy act-table-load pass doesn't thrash sets on every block.
_orig_get_tables = bacc.get_activation_tables


def _reordered_tables(arch):
    t = _orig_get_tables(arch)
    keep = {"natural_log_exp_and_others", "gelu_and_others"}
    return {k: (v if k in keep else set()) for k, v in t.items()}


bacc.get_activation_tables = _reordered_tables

dt = mybir.dt
AF = mybir.ActivationFunctionType
ALU = mybir.AluOpType
DR = mybir.MatmulPerfMode.DoubleRow
BF16 = ml_dtypes.bfloat16
F8 = ml_dtypes.float8_e4m3

B, N, C, H, DH, HID = 4, 1024, 768, 12, 64, 3072
NCORES = 8
OWN = 512          # tokens per core
SCALE = DH ** -0.5
EPS = 1e-5


def round_fp32r(x):
    b = np.ascontiguousarray(x, dtype=np.float32).view(np.uint32)
    lsb = (b >> np.uint32(12)) & np.uint32(1)
    return ((b + np.uint32(0x7FF) + lsb) & np.uint32(0xFFFFF000)).view(np.float32)


# --------------------------------------------------------------------------
# device program
# --------------------------------------------------------------------------

def build_nc(debug=False):
    nc = bacc.Bacc(trn_type="TRN2", target_bir_lowering=False)
    f32 = dt.float32
    f32r = dt.float32r
    bf16 = dt.bfloat16
    f8 = dt.float8e4

    # ---- DRAM I/O ----
    x_r = nc.dram_tensor("x_r", [128, 12, N], f8, kind="ExternalInput")
    x_own = nc.dram_tensor("x_own", [128, 12, OWN], f32, kind="ExternalInput")
    w_qkv = nc.dram_tensor("w_qkv", [H, 128, 12, 384], f8, kind="ExternalInput")
    w_qkv_s = nc.dram_tensor("w_qkv_s", [H, 4, 384], f32r, kind="ExternalInput")
    w_v = nc.dram_tensor("w_v", [6, 128, 12, 256], f8, kind="ExternalInput")
    w_v_s = nc.dram_tensor("w_v_s", [4, 1536], f32r, kind="ExternalInput")
    w_proj = nc.dram_tensor("w_proj", [12, 128, 12, 128], f8, kind="ExternalInput")
    w_pb = nc.dram_tensor("w_pb", [128, 12], f32, kind="ExternalInput")
    w_fc1r = nc.dram_tensor("w_fc1r", [24, 128, 6, 128], bf16, kind="ExternalInput")
    w_fc1i = nc.dram_tensor("w_fc1i", [24, 128, 6, 128], bf16, kind="ExternalInput")
    w_fc1in = nc.dram_tensor("w_fc1in", [24, 128, 6, 128], bf16, kind="ExternalInput")
    w_fc1b = nc.dram_tensor("w_fc1b", [128, 24, 2], f32, kind="ExternalInput")
    w_fc2r = nc.dram_tensor("w_fc2r", [6, 128, 24, 128], bf16, kind="ExternalInput")
    w_fc2i = nc.dram_tensor("w_fc2i", [6, 128, 24, 128], bf16, kind="ExternalInput")
    w_fc2in = nc.dram_tensor("w_fc2in", [6, 128, 24, 128], bf16, kind="ExternalInput")
    w_fc2_s = nc.dram_tensor("w_fc2_s", [6, 4, 256], f32r, kind="ExternalInput")
    ones_col = nc.dram_tensor("ones_col", [128, 1], bf16, kind="ExternalInput")
    ones_ab8 = nc.dram_tensor("ones_ab8", [128, 4], f8, kind="ExternalInput")
    ones_s8 = nc.dram_tensor("ones_s8", [128, 1], f8, kind="ExternalInput")
    ones_ab = nc.dram_tensor("ones_ab", [128, 4], bf16, kind="ExternalInput")
    ones_s = nc.dram_tensor("ones_s", [128, 1], bf16, kind="ExternalInput")
    stat_one = nc.dram_tensor("stat_one", [4, OWN], f32r, kind="ExternalInput")
    ident8 = nc.dram_tensor("ident8", [8, 8], f32r, kind="ExternalInput")

    out_fm = nc.dram_tensor("out_fm", [12, 128, OWN], f32, kind="ExternalOutput")

    with tile.TileContext(nc) as tc, ExitStack() as top:
        consts = top.enter_context(tc.tile_pool(name="consts", bufs=1))
        t_ones_col = consts.tile([128, 1], bf16)
        t_ones_ab8 = consts.tile([128, 4], f8)
        t_ones_s8 = consts.tile([128, 1], f8)
        t_ones_ab = consts.tile([128, 4], bf16)
        t_ones_s = consts.tile([128, 1], bf16)
        t_stat_one = consts.tile([4, OWN], f32r)
        t_id8 = consts.tile([8, 8], f32r)
        t_pb = consts.tile([128, 12], f32)
        t_f1b = consts.tile([128, 24, 2], f32)
        t_eps = consts.tile([1, 1], f32)
        nc.sync.dma_start(t_ones_col[:], ones_col[:])
        nc.sync.dma_start(t_ones_ab8[:], ones_ab8[:])
        nc.sync.dma_start(t_ones_s8[:], ones_s8[:])
        nc.sync.dma_start(t_ones_ab[:], ones_ab[:])
        nc.sync.dma_start(t_ones_s[:], ones_s[:])
        nc.sync.dma_start(t_stat_one[:], stat_one[:])
        nc.sync.dma_start(t_id8[:], ident8[:])
        nc.sync.dma_start(t_pb[:], w_pb[:])
        nc.sync.dma_start(t_f1b[:], w_fc1b[:])
        nc.vector.memset(t_eps[:], EPS)

        poolR1 = top.enter_context(tc.tile_pool(name="poolR1", bufs=1))
        xr1 = poolR1.tile([128, 12, OWN], f32, name="xr1")

        with ExitStack() as es_x:
            poolX = es_x.enter_context(tc.tile_pool(name="poolX", bufs=1))
            x8 = poolX.tile([128, 12, N], f8, name="x8")
            pdram = es_x.enter_context(
                tc.tile_pool(name="pdram", bufs=1, space="DRAM"))
            rstd_dram = pdram.tile([1, N], f32, name="rstd_dram")
            stat1s = [poolX.tile([4, 512], f32r, name=f"stat1_{ch}")
                      for ch in range(2)]
            rstd_bc1s = [poolX.tile([128, 512], f32, name=f"rstd_bc1_{ch}")
                         for ch in range(2)]
            rstdT = poolX.tile([128, 8], f32, name="rstdT")
            for kb in range(12):
                nc.sync.dma_start(x8[:, kb, :], x_r[:, kb, :])

            # ---------------- phase A: LN1 stats over full sequence --------
            with ExitStack() as es_a:
                pa = es_a.enter_context(tc.tile_pool(name="pa_sb", bufs=3))
                pa_ps = es_a.enter_context(
                    tc.tile_pool(name="pa_ps", bufs=2, space="PSUM"))
                pa_sc = es_a.enter_context(tc.tile_pool(name="pa_sc", bufs=2))
                mu_pss = [pa_ps.tile([2, 512], f32, tag=f"mu{ch}",
                                     name=f"mu{ch}", bufs=1) for ch in range(2)]
                s_pss = [pa_ps.tile([1, 512], f32, tag=f"s{ch}",
                                    name=f"s{ch}", bufs=1) for ch in range(2)]
                for kb in range(12):
                    sq = pa.tile([128, N], f8, tag="sq", name=f"sq{kb}")
                    nc.scalar.activation(sq[:], x8[:, kb, :], AF.Square)
                    lhs = t_ones_ab8[:, 0:2] if kb < 6 else t_ones_ab8[:, 2:4]
                    for ch in range(2):
                        sl = slice(ch * 512, ch * 512 + 512)
                        nc.tensor.matmul(mu_pss[ch][:], lhs, x8[:, kb, sl],
                                         start=(kb == 0), stop=(kb == 11))
                        nc.tensor.matmul(s_pss[ch][:], t_ones_s8[:], sq[:, sl],
                                         start=(kb == 0), stop=(kb == 11))
                for ch in range(2):
                    sl = slice(ch * 512, ch * 512 + 512)
                    mu_ps = mu_pss[ch]
                    s_ps = s_pss[ch]
                    # var = S - mu_r^2 - mu_i^2 ; std = exp(.5 ln(var+eps))
                    mu_sb = pa_sc.tile([2, 512], f32, tag="musb", name=f"musb{ch}")
                    var = pa_sc.tile([1, 512], f32, tag="var", name=f"var{ch}")
                    lnv = pa_sc.tile([1, 512], f32, tag="lnv", name=f"lnv{ch}")
                    s_c = pa_sc.tile([1, 512], f32, tag="sc_", name=f"sc_{ch}")
                    sq2p = pa_sc.tile([2, 512], bf16, tag="sq2p",
                                      name=f"sq2p{ch}")
                    nc.vector.tensor_scalar(mu_sb[:], mu_ps[:], 1.0 / C, None,
                                            op0=ALU.mult)
                    nc.vector.tensor_scalar(s_c[:], s_ps[:], 1.0 / C, None,
                                            op0=ALU.mult)
                    nc.vector.tensor_tensor(sq2p[:], mu_sb[:], mu_sb[:],
                                            op=ALU.mult)
                    vps = pa_ps.tile([1, 512], f32, tag=f"vps{ch}",
                                     name=f"vps{ch}", bufs=1)
                    nc.tensor.matmul(vps[:], t_ones_col[0:2, :], sq2p[:],
                                     start=True, stop=True)
                    nc.vector.tensor_tensor(var[:], s_c[:], vps[:],
                                            op=ALU.subtract)
                    nc.scalar.activation(lnv[:], var[:], AF.Ln, bias=t_eps[:])
                    # stats rows: 0=mu_r 1=mu_i 2=std
                    nc.vector.tensor_copy(stat1s[ch][0:2, :], mu_sb[:])
                    std_row = pa_sc.tile([1, 512], f32r, tag="stdr", name=f"stdr{ch}")
                    nc.scalar.activation(std_row[:], lnv[:], AF.Exp, scale=0.5)
                    nc.sync.dma_start(stat1s[ch][2:3, :], std_row[:])
                    rstd_row = pa_sc.tile([1, 512], f32r, tag="rst", name=f"rst{ch}")
                    nc.scalar.activation(rstd_row[:], lnv[:], AF.Exp, scale=-0.5)
                    nc.sync.dma_start(rstd_dram[:, sl], rstd_row[:].bitcast(f32))
                    bcast = bass.AP(tensor=rstd_dram.tensor,
                                    offset=rstd_dram[:, sl].offset,
                                    ap=[[0, 128]] + rstd_dram[:, sl].ap[1:])
                    nc.sync.dma_start(rstd_bc1s[ch][:], bcast)
                # rstd transposed: rstdT[p, t2b] = rstd[t2b*128 + p]
                rstd8 = pa_sc.tile([8, 128], f32, tag="r8", name="rstd8")
                nc.sync.dma_start(
                    rstd8[:], rstd_dram[:].rearrange("o (a b) -> (o a) b", a=8))
                rstdT_ps = pa_ps.tile([128, 8], f32, tag="rtps", name="rtps")
                nc.tensor.transpose(rstdT_ps[:], rstd8[:], t_id8[:].bitcast(f32))
                nc.vector.tensor_copy(rstdT[:], rstdT_ps[:])

            # ---------------- phase BC: qkv + attention per head ----------
            es_attn = ExitStack()
            attnp = es_attn.enter_context(tc.tile_pool(name="attnp", bufs=1))
            attn = attnp.tile([128, 12, OWN], f8, name="attn")
            es_b = ExitStack()
            pq = es_b.enter_context(tc.tile_pool(name="pq", bufs=1))
            pk = es_b.enter_context(tc.tile_pool(name="pk", bufs=1))
            pvt = es_b.enter_context(tc.tile_pool(name="pvt", bufs=2))
            pwv = es_b.enter_context(tc.tile_pool(name="pwv", bufs=1))
            pwq = es_b.enter_context(tc.tile_pool(name="pwq", bufs=2))
            pet = es_b.enter_context(tc.tile_pool(name="pet", bufs=3))
            psc = es_b.enter_context(tc.tile_pool(name="psc", bufs=4))
            prd = es_b.enter_context(tc.tile_pool(name="prd", bufs=2))
            ps_rot = es_b.enter_context(
                tc.tile_pool(name="ps_rot", bufs=2, space="PSUM"))
            ps_sc = es_b.enter_context(
                tc.tile_pool(name="ps_sc", bufs=2, space="PSUM"))
            ps_acc = es_b.enter_context(
                tc.tile_pool(name="ps_acc", bufs=2, space="PSUM"))
            pdram_rd = es_b.enter_context(
                tc.tile_pool(name="pdram_rd", bufs=2, space="DRAM"))
            vt_pair = None
            et_fifo = []
            acc_ps = {}
            LAG = 4

            def emit_avden(ent):
                h2, t2b2, et2, vt2 = ent
                slot2 = h2 % 2
                if t2b2 == 0:
                    acc_ps[h2] = (
                        ps_acc.tile([128, OWN], f32, tag="av", name=f"av{h2}",
                                    bufs=1),
                        ps_acc.tile([1, OWN], f32, tag="den", name=f"den{h2}",
                                    bufs=1),
                    )
                av2, den2 = acc_ps[h2]
                nc.tensor.matmul(den2[:], t_ones_col[:], et2,
                                 start=(t2b2 == 0), stop=(t2b2 == 7))
                dsl2 = slice(slot2 * 128, slot2 * 128 + 128)
                nc.tensor.matmul(av2[:], vt2[:, t2b2, dsl2], et2,
                                 start=(t2b2 == 0), stop=(t2b2 == 7))
                if t2b2 == 7:
                    den_sb = prd.tile([1, OWN], f32, tag="den_sb",
                                      name=f"dsb{h2}", bufs=1)
                    nc.vector.tensor_copy(den_sb[:], den2[:])
                    den_dram = pdram_rd.tile([1, OWN], f32, tag="dend",
                                             name=f"dend{h2}")
                    nc.sync.dma_start(den_dram[:], den_sb[:])
                    den_sp = prd.tile([128, 4], f32, tag="den_sp",
                                      name=f"dsp{h2}", bufs=1)
                    nc.sync.dma_start(
                        den_sp[:],
                        den_dram[:].rearrange("o (a b) -> (o a) b", a=128))
                    rd_sp = prd.tile([128, 4], f32, tag="rd_sp",
                                     name=f"rsp{h2}", bufs=1)
                    nc.vector.reciprocal(rd_sp[:], den_sp[:])
                    rd_dram = pdram_rd.tile([1, OWN], f32, tag="rdd",
                                            name=f"rdd{h2}")
                    nc.sync.dma_start(
                        rd_dram[:].rearrange("o (a b) -> (o a) b", a=128),
                        rd_sp[:])
                    rd_bc = prd.tile([128, OWN], f32, tag="rd_bc",
                                     name=f"rdbc{h2}", bufs=1)
                    rd_bcast_ap = bass.AP(tensor=rd_dram.tensor,
                                          offset=rd_dram[:].offset,
                                          ap=[[0, 128]] + rd_dram[:].ap[1:])
                    nc.sync.dma_start(rd_bc[:], rd_bcast_ap)
                    nc.vector.tensor_tensor(attn[:, h2, :], av2[:], rd_bc[:],
                                            op=ALU.mult)
                    del acc_ps[h2]

            for h in range(H):
                pair, slot = divmod(h, 2)
                # qkv for head h: q1=[q_r;-q_i], q3=[q_i;q_r], k=[k_r;k_i]
                q_t = pq.tile([128, 2, OWN], bf16, tag="q", name=f"q{h}")
                k_t = pk.tile([128, N], bf16, tag="k", name=f"k{h}")
                wqkv_t = pwq.tile([128, 12, 384], f8, tag="wqkv",
                                  name=f"wqkv{h}")
                wqs_t = pwq.tile([4, 384], f32r, tag="wqs", name=f"wqs{h}")
                nc.sync.dma_start(wqkv_t[:], w_qkv[h])
                nc.sync.dma_start(wqs_t[:], w_qkv_s[h])
                q1_ps = ps_rot.tile([128, OWN], f32, tag="rot", name=f"q1ps{h}")
                q3_ps = ps_rot.tile([128, OWN], f32, tag="rot", name=f"q3ps{h}")
                for p in range(6):
                    kp = slice(2 * p, 2 * p + 2)
                    st = (p == 0)
                    nc.tensor.matmul(q1_ps[:], wqkv_t[:, kp, 0:128],
                                     x8[:, kp, 0:OWN], start=st, stop=False,
                                     perf_mode=DR)
                    nc.tensor.matmul(q3_ps[:], wqkv_t[:, kp, 128:256],
                                     x8[:, kp, 0:OWN], start=st, stop=False,
                                     perf_mode=DR)
                nc.tensor.matmul(q1_ps[:], wqs_t[:, 0:128], stat1s[0][:],
                                 start=False, stop=True)
                nc.tensor.matmul(q3_ps[:], wqs_t[:, 128:256], stat1s[0][:],
                                 start=False, stop=True)
                nc.vector.tensor_tensor(q_t[:, 0, :], q1_ps[:],
                                        rstd_bc1s[0][:], op=ALU.mult)
                nc.vector.tensor_tensor(q_t[:, 1, :], q3_ps[:],
                                        rstd_bc1s[0][:], op=ALU.mult)
                k0_ps = ps_rot.tile([128, 512], f32, tag="rot", name=f"k0ps{h}")
                k1_ps = ps_rot.tile([128, 512], f32, tag="rot", name=f"k1ps{h}")
                for p in range(6):
                    kp = slice(2 * p, 2 * p + 2)
                    st = (p == 0)
                    nc.tensor.matmul(k0_ps[:], wqkv_t[:, kp, 256:384],
                                     x8[:, kp, 0:512], start=st, stop=False,
                                     perf_mode=DR)
                    nc.tensor.matmul(k1_ps[:], wqkv_t[:, kp, 256:384],
                                     x8[:, kp, 512:N], start=st, stop=False,
                                     perf_mode=DR)
                nc.tensor.matmul(k0_ps[:], wqs_t[:, 256:384], stat1s[0][:],
                                 start=False, stop=True)
                nc.tensor.matmul(k1_ps[:], wqs_t[:, 256:384], stat1s[1][:],
                                 start=False, stop=True)
                nc.vector.tensor_tensor(k_t[:, 0:512], k0_ps[:],
                                        rstd_bc1s[0][:], op=ALU.mult)
                nc.vector.tensor_tensor(k_t[:, 512:N], k1_ps[:],
                                        rstd_bc1s[1][:], op=ALU.mult)
                if slot == 0:
                    # V^T for this head pair: [t2, d] via swapped operands
                    wv_t = pwv.tile([128, 12, 256], f8, tag="wv",
                                    name=f"wv{pair}")
                    wv_s = pwv.tile([4, 256], f32r, tag="wvs",
                                    name=f"wvs{pair}")
                    csl = slice(pair * 256, pair * 256 + 256)
                    nc.sync.dma_start(wv_t[:], w_v[pair])
                    nc.sync.dma_start(wv_s[:], w_v_s[:, csl])
                    vt_pair = pvt.tile([128, 8, 256], bf16, tag="vt",
                                       name=f"vt{pair}")
                    for t2b in range(8):
                        t2s = slice(t2b * 128, t2b * 128 + 128)
                        vt_ps = ps_rot.tile([128, 256], f32, tag="rot",
                                            name=f"vtps{pair}_{t2b}")
                        for p in range(6):
                            kp = slice(2 * p, 2 * p + 2)
                            nc.tensor.matmul(vt_ps[:], x8[:, kp, t2s],
                                             wv_t[:, kp, :],
                                             start=(p == 0), stop=False,
                                             perf_mode=DR)
                        st1 = stat1s[t2b // 4]
                        t2l = slice((t2b % 4) * 128, (t2b % 4) * 128 + 128)
                        nc.tensor.matmul(vt_ps[:], st1[:, t2l], wv_s[:],
                                         start=False, stop=True)
                        nc.vector.tensor_scalar(
                            vt_pair[:, t2b, :], vt_ps[:],
                            rstdT[:, t2b:t2b + 1], None, op0=ALU.mult)
                # scores + exp chain, batched over block pairs;
                # den/av matmuls lag by LAG sub-blocks
                for t2p in range(4):
                    t2s0 = slice(t2p * 256, t2p * 256 + 128)
                    t2s1 = slice(t2p * 256 + 128, t2p * 256 + 256)
                    sr_pair = ps_sc.tile([128, 2, OWN], f32, tag="scp",
                                         name=f"srp{h}_{t2p}")
                    si_pair = ps_sc.tile([128, 2, OWN], f32, tag="scp",
                                         name=f"sip{h}_{t2p}")
                    nc.tensor.matmul(sr_pair[:, 0, :], k_t[:, t2s0],
                                     q_t[:, 0, :], start=True, stop=True)
                    nc.tensor.matmul(si_pair[:, 0, :], k_t[:, t2s0],
                                     q_t[:, 1, :], start=True, stop=True)
                    nc.tensor.matmul(sr_pair[:, 1, :], k_t[:, t2s1],
                                     q_t[:, 0, :], start=True, stop=True)
                    nc.tensor.matmul(si_pair[:, 1, :], k_t[:, t2s1],
                                     q_t[:, 1, :], start=True, stop=True)
                    sqr = psc.tile([128, 2, OWN], f32, tag="sqr",
                                   name=f"sqr{h}_{t2p}")
                    sqi = psc.tile([128, 2, OWN], f32, tag="sqi",
                                   name=f"sqi{h}_{t2p}")
                    if t2p % 2 == 0:
                        m2q = psc.tile([128, 4, OWN], f32, tag="m2q",
                                       name=f"m2q{h}_{t2p // 2}")
                        etq = pet.tile([128, 4, OWN], bf16, tag="et",
                                       name=f"et{h}_{t2p // 2}")
                    nc.scalar.activation(sqr[:], sr_pair[:], AF.Square)
                    nc.scalar.activation(sqi[:], si_pair[:], AF.Square)
                    qsl = slice(2 * (t2p % 2), 2 * (t2p % 2) + 2)
                    nc.vector.tensor_tensor(m2q[:, qsl, :], sqr[:], sqi[:],
                                            op=ALU.add)
                    if t2p % 2 == 1:
                        # chain over the 4-block quad: ln -> 0.5ln -> mag -> exp
                        nc.scalar.activation(m2q[:], m2q[:], AF.Ln)
                        nc.scalar.activation(m2q[:], m2q[:], AF.Exp, scale=0.5)
                        nc.scalar.activation(etq[:], m2q[:], AF.Exp)
                        for sub in range(4):
                            et_fifo.append((h, (t2p - 1) * 2 + sub,
                                            etq[:, sub, :], vt_pair))
                            while len(et_fifo) > LAG:
                                emit_avden(et_fifo.pop(0))
            for ent in et_fifo:
                emit_avden(ent)
            et_fifo.clear()
            es_b.close()

            # ------------- phase D: proj + residual --------------------
            nc.sync.dma_start(xr1[:], x_own[:])
            for opb in range(12):
                nc.vector.tensor_scalar(xr1[:, opb, :], xr1[:, opb, :],
                                        t_pb[:, opb:opb + 1], None,
                                        op0=ALU.add)
            r1r = poolR1.tile([128, 12, OWN], bf16, name="r1r")
            with ExitStack() as es_d:
                pwp = es_d.enter_context(tc.tile_pool(name="pwp", bufs=3))
                ps_d = es_d.enter_context(
                    tc.tile_pool(name="ps_d", bufs=4, space="PSUM"))
                for opb in range(12):
                    wp_t = pwp.tile([128, 12, 128], f8, tag="wp",
                                    name=f"wp{opb}")
                    nc.sync.dma_start(wp_t[:], w_proj[opb])
                    pr_ps = ps_d.tile([128, OWN], f32, tag="pr",
                                      name=f"prps{opb}")
                    for p in range(6):
                        kp = slice(2 * p, 2 * p + 2)
                        nc.tensor.matmul(pr_ps[:], wp_t[:, kp, :],
                                         attn[:, kp, :],
                                         start=(p == 0), stop=(p == 5),
                                         perf_mode=DR)
                    nc.vector.tensor_tensor(xr1[:, opb, :], pr_ps[:],
                                            xr1[:, opb, :], op=ALU.add)
                    nc.vector.tensor_copy(r1r[:, opb, :], xr1[:, opb, :])
            es_attn.close()

        # ---------------- phase E: LN2 stats + normalized r1 --------------
        poolE = top.enter_context(tc.tile_pool(name="poolE", bufs=1))
        xh2 = poolE.tile([128, 12, OWN], bf16, name="xh2")
        with ExitStack() as es_e:
            pe = es_e.enter_context(tc.tile_pool(name="pe_sb", bufs=1))
            pdram2 = es_e.enter_context(
                tc.tile_pool(name="pdram2", bufs=1, space="DRAM"))
            pe_ps = es_e.enter_context(
                tc.tile_pool(name="pe_ps", bufs=2, space="PSUM"))
            sq2s = []
            for kb in range(12):
                sq2 = pe.tile([128, OWN], bf16, tag="sq2", name=f"sq2_{kb}", bufs=12)
                nc.scalar.activation(sq2[:], r1r[:, kb, :], AF.Square)
                sq2s.append(sq2)
            mu2_ps = pe_ps.tile([2, OWN], f32, tag="mu2", name="mu2")
            s2_ps = pe_ps.tile([1, OWN], f32, tag="s2", name="s2")
            for kb in range(12):
                lhs = t_ones_ab[:, 0:2] if kb < 6 else t_ones_ab[:, 2:4]
                nc.tensor.matmul(mu2_ps[:], lhs, r1r[:, kb, :],
                                 start=(kb == 0), stop=(kb == 11))
                nc.tensor.matmul(s2_ps[:], t_ones_s[:], sq2s[kb][:],
                                 start=(kb == 0), stop=(kb == 11))
            mu2_sb = pe.tile([2, OWN], f32, tag="emusb", name="emusb")
            var = pe.tile([1, OWN], f32, tag="evar", name="evar")
            lnv = pe.tile([1, OWN], f32, tag="elnv", name="elnv")
            s2_c = pe.tile([1, OWN], f32, tag="es2c", name="es2c")
            sq2p2 = pe.tile([2, OWN], bf16, tag="esq2p", name="esq2p")
            nc.vector.tensor_scalar(mu2_sb[:], mu2_ps[:], 1.0 / C, None,
                                    op0=ALU.mult)
            nc.vector.tensor_scalar(s2_c[:], s2_ps[:], 1.0 / C, None,
                                    op0=ALU.mult)
            nc.vector.tensor_tensor(sq2p2[:], mu2_sb[:], mu2_sb[:],
                                    op=ALU.mult)
            vps2 = pe_ps.tile([1, OWN], f32, tag="vps2", name="vps2", bufs=1)
            nc.tensor.matmul(vps2[:], t_ones_col[0:2, :], sq2p2[:],
                             start=True, stop=True)
            nc.vector.tensor_tensor(var[:], s2_c[:], vps2[:],
                                    op=ALU.subtract)
            nc.scalar.activation(lnv[:], var[:], AF.Ln, bias=t_eps[:])
            rstd2_row = pe.tile([1, OWN], f32, tag="ers", name="ers")
            nc.scalar.activation(rstd2_row[:], lnv[:], AF.Exp, scale=-0.5)
            stat2_dram = pdram2.tile([4, OWN], f32, name="stat2_dram")
            nc.sync.dma_start(stat2_dram[0:1, :], mu2_sb[0:1, :])
            nc.sync.dma_start(stat2_dram[1:2, :], mu2_sb[1:2, :])
            nc.sync.dma_start(stat2_dram[2:3, :], rstd2_row[:])
            mu2r_bc = pe.tile([128, OWN], f32, tag="m2rbc", name="m2rbc")
            mu2i_bc = pe.tile([128, OWN], f32, tag="m2ibc", name="m2ibc")
            rstd2_bc = pe.tile([128, OWN], f32, tag="r2bc", name="r2bc")
            for row, t in ((0, mu2r_bc), (1, mu2i_bc), (2, rstd2_bc)):
                rr = stat2_dram[row:row + 1, :]
                nc.sync.dma_start(t[:], bass.AP(
                    tensor=stat2_dram.tensor, offset=rr.offset,
                    ap=[[0, 128]] + rr.ap[1:]))
            for kb in range(12):
                mbc = mu2r_bc if kb < 6 else mu2i_bc
                nc.vector.tensor_tensor(xh2[:, kb, :], r1r[:, kb, :],
                                        mbc[:], op=ALU.subtract)
                nc.vector.tensor_tensor(xh2[:, kb, :], xh2[:, kb, :],
                                        rstd2_bc[:], op=ALU.mult)

        # ---------------- phase F: MLP, single 512-token pass -------------
        with ExitStack() as es_f:
            ph = es_f.enter_context(tc.tile_pool(name="ph", bufs=1))
            pw1 = es_f.enter_context(tc.tile_pool(name="pw1", bufs=3))
            pw2 = es_f.enter_context(tc.tile_pool(name="pw2", bufs=3))
            pout = es_f.enter_context(tc.tile_pool(name="pout", bufs=2))
            ps_f = es_f.enter_context(
                tc.tile_pool(name="ps_f", bufs=4, space="PSUM"))
            h_t = ph.tile([128, 48, OWN], bf16, name="h_t")
            for Cb in range(24):
                w1r_t = pw1.tile([128, 6, 128], bf16, tag="w1r",
                                 name=f"w1r{Cb}")
                w1i_t = pw1.tile([128, 6, 128], bf16, tag="w1i",
                                 name=f"w1i{Cb}")
                w1in_t = pw1.tile([128, 6, 128], bf16, tag="w1in",
                                  name=f"w1in{Cb}")
                nc.scalar.dma_start(w1r_t[:], w_fc1r[Cb])
                nc.scalar.dma_start(w1i_t[:], w_fc1i[Cb])
                nc.scalar.dma_start(w1in_t[:], w_fc1in[Cb])
                hr_ps = ps_f.tile([128, OWN], f32, tag="fps",
                                  name=f"hrps{Cb}")
                hi_ps = ps_f.tile([128, OWN], f32, tag="fps",
                                  name=f"hips{Cb}")
                for kb in range(6):
                    st = (kb == 0)
                    nc.tensor.matmul(hr_ps[:], w1r_t[:, kb, :],
                                     xh2[:, kb, :], start=st, stop=False)
                    nc.tensor.matmul(hi_ps[:], w1i_t[:, kb, :],
                                     xh2[:, kb, :], start=st, stop=False)
                for kb in range(6):
                    lst = (kb == 5)
                    nc.tensor.matmul(hr_ps[:], w1in_t[:, kb, :],
                                     xh2[:, 6 + kb, :], start=False,
                                     stop=lst)
                    nc.tensor.matmul(hi_ps[:], w1r_t[:, kb, :],
                                     xh2[:, 6 + kb, :], start=False,
                                     stop=lst)
                nc.scalar.activation(h_t[:, Cb, :], hr_ps[:], AF.Gelu,
                                     bias=t_f1b[:, Cb, 0:1])
                nc.scalar.activation(h_t[:, 24 + Cb, :], hi_ps[:], AF.Gelu,
                                     bias=t_f1b[:, Cb, 1:2])
            for j in range(6):
                w2r_t = pw2.tile([128, 24, 128], bf16, tag="w2r",
                                 name=f"w2r{j}")
                w2i_t = pw2.tile([128, 24, 128], bf16, tag="w2i",
                                 name=f"w2i{j}")
                w2in_t = pw2.tile([128, 24, 128], bf16, tag="w2in",
                                  name=f"w2in{j}")
                w2s_t = pw2.tile([4, 256], f32r, tag="w2s",
                                 name=f"w2s{j}")
                nc.scalar.dma_start(w2r_t[:], w_fc2r[j])
                nc.scalar.dma_start(w2i_t[:], w_fc2i[j])
                nc.scalar.dma_start(w2in_t[:], w_fc2in[j])
                nc.sync.dma_start(w2s_t[:], w_fc2_s[j])
                or_ps = ps_f.tile([128, OWN], f32, tag="fps",
                                  name=f"orps{j}")
                oi_ps = ps_f.tile([128, OWN], f32, tag="fps",
                                  name=f"oips{j}")
                for kb in range(24):
                    st = (kb == 0)
                    nc.tensor.matmul(or_ps[:], w2r_t[:, kb, :], h_t[:, kb, :],
                                     start=st, stop=False)
                    nc.tensor.matmul(oi_ps[:], w2i_t[:, kb, :], h_t[:, kb, :],
                                     start=st, stop=False)
                for kb in range(24):
                    nc.tensor.matmul(or_ps[:], w2in_t[:, kb, :],
                                     h_t[:, 24 + kb, :],
                                     start=False, stop=False)
                    nc.tensor.matmul(oi_ps[:], w2r_t[:, kb, :],
                                     h_t[:, 24 + kb, :],
                                     start=False, stop=False)
                nc.tensor.matmul(or_ps[:], w2s_t[:, 0:128],
                                 t_stat_one[:], start=False, stop=True)
                nc.tensor.matmul(oi_ps[:], w2s_t[:, 128:256],
                                 t_stat_one[:], start=False, stop=True)
                o_r = pout.tile([128, OWN], f32, tag="o", name=f"or{j}")
                o_i = pout.tile([128, OWN], f32, tag="o", name=f"oi{j}")
                nc.vector.tensor_tensor(o_r[:], or_ps[:], xr1[:, j, :],
                                        op=ALU.add)
                nc.vector.tensor_tensor(o_i[:], oi_ps[:], xr1[:, 6 + j, :],
                                        op=ALU.add)
                nc.sync.dma_start(out_fm[j], o_r[:])
                nc.sync.dma_start(out_fm[6 + j], o_i[:])
    nc.compile()
    return nc


# --------------------------------------------------------------------------
# host side
# --------------------------------------------------------------------------

def _cx(a):
    return a[..., 0].astype(np.float64) + 1j * a[..., 1].astype(np.float64)


def _kcols(Wp, wsum, wb, plane, scale=1.0):
    """K-profile [1539, m] for output features with complex weight rows Wp
    [m, 768], LN fold sums wsum [m], bias-column wb [m]. K rows: xr(768),
    xi(768), mu_r, mu_i, std."""
    m = Wp.shape[0]
    out = np.zeros((1539, m), np.float64)
    if plane == "r":
        out[0:768] = Wp.real.T
        out[768:1536] = -Wp.imag.T
        out[1536] = -wsum.real
        out[1537] = wsum.imag
        out[1538] = wb.real
    else:
        out[0:768] = Wp.imag.T
        out[768:1536] = Wp.real.T
        out[1536] = -wsum.imag
        out[1537] = -wsum.real
        out[1538] = wb.imag
    return out * scale


def _bf(a):
    return np.ascontiguousarray(a).astype(BF16)


def _f8(a):
    return np.ascontiguousarray(a).astype(F8)


def _pmajor(a):
    """[kb, 128, n] -> [128, kb, n] partition-major contiguous."""
    return np.ascontiguousarray(np.transpose(a, (1, 0, 2)))


def _prep_weights(inputs):
    n1 = _cx(inputs["n1_w"]); b1 = _cx(inputs["n1_b"])
    n2 = _cx(inputs["n2_w"]); b2 = _cx(inputs["n2_b"])
    Wqkv = _cx(inputs["qkv_w"])          # [2304, 768]
    Wp = _cx(inputs["proj_w"])           # [768, 768]
    bp = _cx(inputs["proj_b"])           # [768]
    W1 = _cx(inputs["fc1_w"])            # [3072, 768]
    bf1 = _cx(inputs["fc1_b"])           # [3072]
    W2 = _cx(inputs["fc2_w"])            # [768, 3072]
    bf2 = _cx(inputs["fc2_b"])           # [768]

    d = {}
    # ---- qkv (LN1-folded) ----
    Wq, Wk, Wv = Wqkv[0:768], Wqkv[768:1536], Wqkv[1536:2304]

    def fold1(W):
        Wf = W * n1[None, :]
        return Wf, Wf.sum(1), W @ b1

    w_qkv = np.zeros((H, 128, 12, 384), F8)
    w_qkv_s = np.zeros((H, 4, 384), np.float32)
    for h in range(H):
        rows = slice(h * DH, (h + 1) * DH)
        Qf, Qs, Qb = fold1(Wq[rows])
        Kf, Ks, Kb_ = fold1(Wk[rows])
        q1 = np.hstack([_kcols(Qf, Qs, Qb, "r", SCALE),
                        _kcols(Qf, Qs, Qb, "i", -SCALE)])
        q3 = np.hstack([_kcols(Qf, Qs, Qb, "i", SCALE),
                        _kcols(Qf, Qs, Qb, "r", SCALE)])
        kk = np.hstack([_kcols(Kf, Ks, Kb_, "r"), _kcols(Kf, Ks, Kb_, "i")])
        blk = np.hstack([q1, q3, kk]).astype(np.float32)       # [1539, 384]
        w_qkv[h] = _f8(_pmajor(blk[0:1536].reshape(12, 128, 384)))
        w_qkv_s[h, 0:3] = blk[1536:1539]
    d["w_qkv"] = w_qkv
    d["w_qkv_s"] = round_fp32r(w_qkv_s)

    # ---- v (LN1-folded), rhs layout; cols: pair*256+slot*128+plane*64+dh
    wv_full = np.zeros((1539, 1536), np.float64)
    for h in range(H):
        rows = slice(h * DH, (h + 1) * DH)
        Vf, Vs, Vb = fold1(Wv[rows])
        base = h * 128
        wv_full[:, base:base + 64] = _kcols(Vf, Vs, Vb, "r")
        wv_full[:, base + 64:base + 128] = _kcols(Vf, Vs, Vb, "i")
    w_v = np.zeros((6, 128, 12, 256), F8)
    for pair in range(6):
        csl = slice(pair * 256, pair * 256 + 256)
        w_v[pair] = _f8(_pmajor(wv_full[0:1536, csl].reshape(12, 128, 256)))
    d["w_v"] = w_v
    wvs = np.zeros((4, 1536), np.float32)
    wvs[0:3] = wv_full[1536:1539]
    d["w_v_s"] = round_fp32r(wvs)

    # ---- proj; K rows = attn features: per head [a_r(64); a_i(64)] ----
    w_proj = np.zeros((12, 128, 12, 128), F8)
    w_pb = np.zeros((128, 12), np.float32)
    for opb in range(12):
        plane = "r" if opb < 6 else "i"
        orow = slice((opb % 6) * 128, (opb % 6) * 128 + 128)
        Wpo = Wp[orow]                               # [128, 768] complex
        prof = np.zeros((1536, 128), np.float64)
        for hh in range(H):
            cols = slice(hh * DH, (hh + 1) * DH)
            if plane == "r":
                prof[hh * 128:hh * 128 + 64] = Wpo.real[:, cols].T
                prof[hh * 128 + 64:hh * 128 + 128] = -Wpo.imag[:, cols].T
            else:
                prof[hh * 128:hh * 128 + 64] = Wpo.imag[:, cols].T
                prof[hh * 128 + 64:hh * 128 + 128] = Wpo.real[:, cols].T
        w_proj[opb] = _f8(_pmajor(prof.reshape(12, 128, 128)))
        w_pb[:, opb] = (bp.real if plane == "r" else bp.imag)[orow]
    d["w_proj"] = w_proj
    d["w_pb"] = w_pb

    # ---- fc1 (gain-folded; bias separate; LN2 applied via xh2) ----
    W1f = W1 * n2[None, :]
    W1b = W1 @ b2 + bf1
    w_fc1r = np.zeros((24, 128, 6, 128), BF16)
    w_fc1i = np.zeros((24, 128, 6, 128), BF16)
    w_fc1in = np.zeros((24, 128, 6, 128), BF16)
    w_fc1b = np.zeros((128, 24, 2), np.float32)
    for Cb in range(24):
        orow = slice(Cb * 128, (Cb + 1) * 128)
        tr = np.zeros((6, 128, 128), np.float64)
        ti = np.zeros((6, 128, 128), np.float64)
        for kb in range(6):
            icol = slice(kb * 128, (kb + 1) * 128)
            tr[kb] = W1f.real[orow, icol].T
            ti[kb] = W1f.imag[orow, icol].T
        w_fc1r[Cb] = _bf(_pmajor(tr))
        w_fc1i[Cb] = _bf(_pmajor(ti))
        w_fc1in[Cb] = _bf(_pmajor(-ti))
        w_fc1b[:, Cb, 0] = W1b.real[orow]
        w_fc1b[:, Cb, 1] = W1b.imag[orow]
    d["w_fc1r"] = w_fc1r
    d["w_fc1i"] = w_fc1i
    d["w_fc1in"] = w_fc1in
    d["w_fc1b"] = w_fc1b

    # ---- fc2 (plain + bias) ----
    w_fc2r = np.zeros((6, 128, 24, 128), BF16)
    w_fc2i = np.zeros((6, 128, 24, 128), BF16)
    w_fc2in = np.zeros((6, 128, 24, 128), BF16)
    w_fc2_s = np.zeros((6, 4, 256), np.float32)
    for j in range(6):
        orow = slice(j * 128, (j + 1) * 128)
        tr = np.zeros((24, 128, 128), np.float64)
        ti = np.zeros((24, 128, 128), np.float64)
        for kb in range(24):
            icol = slice(kb * 128, (kb + 1) * 128)
            tr[kb] = W2.real[orow, icol].T
            ti[kb] = W2.imag[orow, icol].T
        w_fc2r[j] = _bf(_pmajor(tr))
        w_fc2i[j] = _bf(_pmajor(ti))
        w_fc2in[j] = _bf(_pmajor(-ti))
        w_fc2_s[j, 0, 0:128] = bf2.real[orow]
        w_fc2_s[j, 0, 128:256] = bf2.imag[orow]
    d["w_fc2r"] = w_fc2r
    d["w_fc2i"] = w_fc2i
    d["w_fc2in"] = w_fc2in
    d["w_fc2_s"] = round_fp32r(w_fc2_s)

    # ---- consts ----
    d["ones_col"] = np.ones((128, 1), BF16)
    oab = np.zeros((128, 4), np.float32)
    oab[:, 0] = 1.0
    oab[:, 3] = 1.0
    d["ones_ab8"] = oab.astype(F8)
    d["ones_s8"] = np.ones((128, 1), F8)
    d["ones_ab"] = oab.astype(BF16)
    d["ones_s"] = np.ones((128, 1), BF16)
    so = np.zeros((4, OWN), np.float32)
    so[0] = 1.0
    d["stat_one"] = so
    d["ident8"] = np.eye(8, dtype=np.float32)
    return d


_NC_CACHE = {}


def kernel(**inputs):
    debug = bool(inputs.pop("_debug", False))
    if debug not in _NC_CACHE:
        _NC_CACHE[debug] = build_nc(debug=debug)
    nc = _NC_CACHE[debug]

    shared = _prep_weights(inputs)
    x = np.asarray(inputs["x"], np.float32)          # [B, N, C, 2]

    in_maps = []
    for c in range(NCORES):
        b, half = divmod(c, 2)
        xr_ = x[b, :, :, 0].T                        # [768, 1024]
        xi_ = x[b, :, :, 1].T
        stack = np.concatenate([xr_, xi_], 0)        # [1536, 1024]
        if half == 1:
            stack = np.concatenate([stack[:, OWN:], stack[:, :OWN]], 1)
        m = dict(shared)
        m["x_r"] = np.ascontiguousarray(
            stack.reshape(12, 128, N).transpose(1, 0, 2)).astype(F8)
        m["x_own"] = np.ascontiguousarray(
            stack[:, 0:OWN].reshape(12, 128, OWN).transpose(1, 0, 2))
        in_maps.append(m)

    res = run_bass_kernel_spmd(nc, in_maps, list(range(NCORES)))
    out = np.empty((B, N, C, 2), np.float32)
    for c in range(NCORES):
        b, half = divmod(c, 2)
        o = res.results[c]["out_fm"]                 # [12, 128, OWN]
        sl = slice(half * OWN, half * OWN + OWN)
        out[b, sl, :, 0] = o[0:6].reshape(768, OWN).T
        out[b, sl, :, 1] = o[6:12].reshape(768, OWN).T
    if debug:
        return out, res
    return out
